# revision 1
# baseline (speedup 1.0000x reference)
"""Trainium2 Bass kernel for nn_CustomDeepseekDBOModel (DeepSeek-style MoE layer).

Strategy (8 NeuronCores, expert-parallel):
  * Every core receives the full token set (T=1024 is small) plus its own
    shard of the routed-expert weights (4 of 32 experts) and a TP slice of
    the shared expert (1/8 of the intermediate dim).
  * Gating (softmax + group-limited top-k) is computed on every core in
    near-fp32 precision (hi/lo bf16 split matmuls).
  * Each core gathers the tokens routed to its 4 local experts with
    `dma_gather` (no inter-core dispatch traffic at all), runs the expert
    SwiGLU MLPs in bf16, scales rows by the routing weights, and gathers
    them back per token with a second `dma_gather`.
  * Routed partial + shared-expert partial accumulate into a transposed
    [H, T] fp32 buffer; a ReduceScatter sums across cores and each core
    emits its H-chunk. The host stitches chunks and transposes.

kernel(**inputs) takes the FULL unsharded inputs and returns the full
[T, H] float32 output.
"""

from contextlib import ExitStack
from dataclasses import dataclass

import ml_dtypes
import numpy as np

import concourse.bass as bass  # noqa: F401  (kept for callers/debugging)
import concourse.mybir as mybir
import concourse.tile as tile
from concourse import bacc

F32 = mybir.dt.float32
BF16 = mybir.dt.bfloat16
I16 = mybir.dt.int16
U32 = mybir.dt.uint32
NPBF16 = ml_dtypes.bfloat16

AF = mybir.ActivationFunctionType
ALU = mybir.AluOpType
AX = mybir.AxisListType


@dataclass(frozen=True)
class Cfg:
    T: int = 1024          # tokens
    H: int = 2048          # hidden
    E: int = 32            # routed experts
    K: int = 6             # top-k
    G: int = 8             # routing groups
    TKG: int = 3           # top-k groups
    IM: int = 768          # moe intermediate
    ISH: int = 1536        # shared intermediate (n_shared * IM)
    NC: int = 8            # cores
    CAP: int = 256         # internal per-expert capacity (multiple of 128)
    SCALE: float = 16.0
    no_collective: bool = False  # replace RS with a local copy (cost model)

    @property
    def EL(self):
        return self.E // self.NC

    @property
    def TT(self):
        return self.T // 128

    @property
    def HK(self):
        return self.H // 128

    @property
    def IM2(self):
        return 2 * self.IM

    @property
    def IMK(self):
        return self.IM // 128

    @property
    def CAPC(self):
        return self.CAP // 128

    @property
    def NSLOT(self):
        return self.EL * self.CAP

    @property
    def NRANK(self):
        return self.NSLOT // 128 + 1

    @property
    def ISHL(self):
        return self.ISH // self.NC

    @property
    def HO(self):
        return self.H // self.NC


FULL = Cfg()

BIGP = 1 << 14  # mask value pushed onto invalid positions


def _chunks(n, step=128):
    out = []
    o = 0
    while o < n:
        out.append((o, min(step, n - o)))
        o += step
    return out


# ---------------------------------------------------------------------------
# device program
# ---------------------------------------------------------------------------


def build_nc(cfg: Cfg):
    c = cfg
    nc = bacc.Bacc("TRN2", target_bir_lowering=False, debug=False,
                   num_devices=c.NC)

    def inp(name, shape, dt):
        return nc.dram_tensor(name, list(shape), dt, kind="ExternalInput")

    tn = {}
    tn["xrow"] = inp("xrow", (c.T, c.H), BF16)
    tn["xhiT"] = inp("xhiT", (c.H, c.T), BF16)
    tn["xloT"] = inp("xloT", (c.H, c.T), BF16)
    tn["gwhiT"] = inp("gwhiT", (c.H, c.E), BF16)
    tn["gwloT"] = inp("gwloT", (c.H, c.E), BF16)
    tn["w13T"] = inp("w13T", (c.EL, c.H, c.IM2), BF16)
    tn["w2T"] = inp("w2T", (c.EL, c.IM, c.H), BF16)
    tn["sguT"] = inp("sguT", (c.H, 2 * c.ISHL), BF16)
    tn["sdnT"] = inp("sdnT", (c.ISHL, c.H), BF16)
    tn["c_t1"] = inp("c_t1", (128, 128), F32)
    tn["c_ones"] = inp("c_ones", (128, 128), F32)
    tn["c_ident"] = inp("c_ident", (128, 128), F32)
    tn["c_iota_tok"] = inp("c_iota_tok", (128, c.TT), F32)
    tn["c_iota_slot"] = inp("c_iota_slot", (128, c.CAP), F32)
    tn["c_iota_g"] = inp("c_iota_g", (128, c.E), F32)
    tn["c_iota_eloc"] = inp("c_iota_eloc", (128, c.EL), F32)
    tn["c_ebase"] = inp("c_ebase", (128, c.EL), F32)

    # rs_in holds y^T in [partition, h-chunk, token] layout (h = hc*128 + p);
    # the reduce-scatter chunks it by partition blocks of 128/NC.
    OUTP = 128 // c.NC
    tn["out_ext"] = nc.dram_tensor("out", [OUTP, c.HK * c.T], F32,
                                   kind="ExternalOutput")
    HW2 = c.HK * c.T // 2
    tn["rs_in0"] = nc.dram_tensor("rs_in0", [128, HW2], F32)
    tn["rs_in1"] = nc.dram_tensor("rs_in1", [128, HW2], F32)
    tn["rs_out0"] = nc.dram_tensor("rs_out0", [OUTP, HW2], F32)
    tn["rs_out1"] = nc.dram_tensor("rs_out1", [OUTP, HW2], F32)
    tn["y_dram"] = nc.dram_tensor("y_dram", [c.NSLOT + 1, c.H], BF16)
    tn["idxd_dram"] = nc.dram_tensor("idxd_dram", [c.NSLOT], I16)
    tn["idxc_dram"] = nc.dram_tensor("idxc_dram", [c.T * c.EL], I16)

    with tile.TileContext(nc) as tc:
        _build_body(nc, tc, c, tn)
    nc.compile()
    return nc


def _build_body(nc, tc, c: Cfg, tn):
    xrow = tn["xrow"]; xhiT = tn["xhiT"]; xloT = tn["xloT"]
    gwhiT = tn["gwhiT"]; gwloT = tn["gwloT"]
    w13T = tn["w13T"]; w2T = tn["w2T"]; sguT = tn["sguT"]; sdnT = tn["sdnT"]
    rs_in = [tn["rs_in0"], tn["rs_in1"]]
    rs_out = [tn["rs_out0"], tn["rs_out1"]]
    out_ext = tn["out_ext"]
    idxd_dram = tn["idxd_dram"]; idxc_dram = tn["idxc_dram"]
    y_dram = tn["y_dram"]

    NK = c.K
    HH = c.H // 2                  # H half
    HB = HH // 128                 # h-chunks per half

    with ExitStack() as es:
        P = es.enter_context(tc.tile_pool(name="persist", bufs=1))

        def load_const(t, shape, tag):
            tl = P.tile(list(shape), F32, tag=tag)
            nc.sync.dma_start(out=tl[:], in_=t.ap())
            return tl

        t1 = load_const(tn["c_t1"], (128, 128), "t1")
        ones = load_const(tn["c_ones"], (128, 128), "ones")
        ident = load_const(tn["c_ident"], (128, 128), "ident")
        iota_tok = load_const(tn["c_iota_tok"], (128, c.TT), "iota_tok")
        iota_slot = load_const(tn["c_iota_slot"], (128, c.CAP), "iota_slot")
        iota_g = load_const(tn["c_iota_g"], (128, c.E), "iota_g")
        iota_eloc = load_const(tn["c_iota_eloc"], (128, c.EL), "iota_eloc")
        ebase = load_const(tn["c_ebase"], (128, c.EL), "ebase")

        xhiT_k = []
        for kc in range(c.HK):
            t = P.tile([128, c.T], BF16, tag=f"xhiT{kc}", name=f"xhiT{kc}")
            nc.sync.dma_start(out=t[:],
                              in_=xhiT.ap()[kc * 128:(kc + 1) * 128, :])
            xhiT_k.append(t)

        # zero row of the DRAM y store (gathered for non-local slots)
        zrow = P.tile([1, c.H], BF16, tag="zrow", name="zrow")
        nc.vector.memset(zrow[:], 0.0)
        nc.sync.dma_start(out=y_dram.ap()[c.NSLOT:c.NSLOT + 1, :],
                          in_=zrow[:])

        # shared-expert gate/up runs early: it only needs xhiT and sguT and
        # fills the PE while the routing chain occupies DVE/ACT.
        g_tiles = _chunks(c.ISHL)
        sgk = []
        for kc in range(c.HK):
            t = P.tile([128, 2 * c.ISHL], BF16, tag=f"sgk{kc}", name=f"sgk{kc}")
            nc.sync.dma_start(out=t[:],
                              in_=sguT.ap()[kc * 128:(kc + 1) * 128, :])
            sgk.append(t)
        sdn_tiles = []
        for gi, (ko, kh) in enumerate(g_tiles):
            t = P.tile([kh, c.H], BF16, tag=f"sdnt{gi}", name=f"sdnt{gi}")
            nc.sync.dma_start(out=t[:], in_=sdnT.ap()[ko:ko + kh, :])
            sdn_tiles.append(t)
        actsh = []
        with tc.tile_pool(name="ps_sh", bufs=2, space="PSUM") as PSSH, \
                tc.tile_pool(name="sgshp", bufs=2) as SGSH:
            for gi, (mo, mh) in enumerate(g_tiles):
                at = P.tile([mh, c.T], BF16, tag=f"actsh{gi}",
                            name=f"actsh{gi}")
                for no, nh in _chunks(c.T, 512):
                    gps = PSSH.tile([128, 512], F32, tag="gsh_ps",
                                    name="gsh_ps")
                    ups = PSSH.tile([128, 512], F32, tag="gsh_ps",
                                    name="gsh_ps")
                    for pso, tgt in ((mo, gps), (c.ISHL + mo, ups)):
                        for kc in range(c.HK):
                            nc.tensor.matmul(
                                tgt[:mh, :nh],
                                sgk[kc][:, pso:pso + mh],
                                xhiT_k[kc][:, no:no + nh],
                                start=(kc == 0), stop=(kc == c.HK - 1))
                    sg = SGSH.tile([128, 512], F32, tag="sgsh", name="sgsh")
                    nc.scalar.activation(sg[:mh, :nh], gps[:mh, :nh],
                                         AF.Sigmoid)
                    nc.vector.tensor_tensor(sg[:mh, :nh], sg[:mh, :nh],
                                            gps[:mh, :nh], op=ALU.mult)
                    nc.vector.tensor_tensor(at[:, no:no + nh], sg[:mh, :nh],
                                            ups[:mh, :nh], op=ALU.mult)
                actsh.append(at)

        posm_sb = P.tile([128, c.TT, c.EL], F32, tag="posm_sb", name="posm_sb")
        woh_sb = P.tile([128, c.TT, c.EL], F32, tag="woh_sb", name="woh_sb")
        wslot_sb = P.tile([128, c.EL * c.CAPC], F32, tag="wslot_sb", name="wslot_sb")
        idxd_sb = [P.tile([128, c.CAP // 16], I16, tag=f"idxd{el}", name=f"idxd{el}")
                   for el in range(c.EL)]
        idxc_sb = [P.tile([128, 8 * c.EL], I16, tag=f"idxc{tt}", name=f"idxc{tt}")
                   for tt in range(c.TT)]

        # =================================================================
        # Phase A: gating + routing
        # =================================================================
        with tc.tile_pool(name="gate", bufs=1) as GP, \
                tc.tile_pool(name="gate2", bufs=2) as G2, \
                tc.tile_pool(name="ps_gate", bufs=2, space="PSUM") as PSG, \
                tc.tile_pool(name="ps_tp", bufs=2, space="PSUM") as PST:
            xloT_k = []
            for kc in range(c.HK):
                t = GP.tile([128, c.T], BF16, tag=f"xloT{kc}", name=f"xloT{kc}")
                nc.sync.dma_start(out=t[:],
                                  in_=xloT.ap()[kc * 128:(kc + 1) * 128, :])
                xloT_k.append(t)
            gwhi_sb = GP.tile([128, c.HK, c.E], BF16, tag="gwhi", name="gwhi")
            nc.sync.dma_start(
                out=gwhi_sb[:],
                in_=gwhiT.ap().rearrange("(k p) e -> p k e", p=128))
            gwlo_sb = GP.tile([128, c.HK, c.E], BF16, tag="gwlo", name="gwlo")
            nc.sync.dma_start(
                out=gwlo_sb[:],
                in_=gwloT.ap().rearrange("(k p) e -> p k e", p=128))

            # logits^T [E, T] in near-fp32 (hi/lo split)
            lgT = GP.tile([c.E, c.T], F32, tag="lgT", name="lgT")
            for no, nh in _chunks(c.T, 512):
                ps = PSG.tile([c.E, 512], F32, tag="lgT_ps", name="lgT_ps")
                for kc in range(c.HK):
                    pairs = [(gwhi_sb[:, kc, :], xhiT_k[kc]),
                             (gwlo_sb[:, kc, :], xhiT_k[kc]),
                             (gwhi_sb[:, kc, :], xloT_k[kc])]
                    for j, (lhsT, rhs) in enumerate(pairs):
                        nc.tensor.matmul(
                            ps[:, :nh], lhsT, rhs[:, no:no + nh],
                            start=(kc == 0 and j == 0),
                            stop=(kc == c.HK - 1 and j == 2))
                nc.scalar.copy(lgT[:, no:no + nh], ps[:, :nh])

            oh_sb = GP.tile([128, c.TT, c.EL], F32, tag="oh_sb", name="oh_sb")

            for tt in range(c.TT):
                tsl = slice(tt * 128, (tt + 1) * 128)
                lg_ps = PST.tile([128, c.E], F32, tag="lg_ps", name="lg_ps")
                nc.tensor.transpose(lg_ps[:], lgT[:, tsl],
                                    ident[:c.E, :c.E])
                lg = G2.tile([128, c.E], F32, tag="lg", name="lg")
                nc.vector.tensor_copy(lg[:], lg_ps[:])

                # softmax numerator (|logits| < ~6, no max-subtraction
                # needed in fp32); selection is scale-invariant so only the
                # top-k weights get normalized.
                exps = G2.tile([128, c.E], F32, tag="exps", name="exps")
                sums = G2.tile([128, 1], F32, tag="sums", name="sums")
                nc.scalar.activation(exps[:], lg[:], AF.Exp,
                                     scale=1.0, accum_out=sums[:])
                rec = G2.tile([128, 1], F32, tag="rec", name="rec")
                nc.vector.reciprocal(rec[:], sums[:])

                # group-limited mask (on gpsimd; DVE runs the main chain)
                gsc = G2.tile([128, c.G], F32, tag="gsc", name="gsc")
                nc.vector.tensor_reduce(
                    gsc[:], exps[:].rearrange("p (g r) -> p g r", g=c.G),
                    AX.X, ALU.max)
                gmax = G2.tile([128, 8], F32, tag="gmax", name="gmax")
                gidx = G2.tile([128, 8], U32, tag="gidx", name="gidx")
                nc.vector.max_with_indices(gmax[:], gidx[:], gsc[:])
                gidxf = G2.tile([128, c.TKG], F32, tag="gidxf", name="gidxf")
                nc.gpsimd.tensor_copy(gidxf[:], gidx[:, :c.TKG])
                smask = G2.tile([128, c.E], F32, tag="smask", name="smask")
                eqg = G2.tile([128, c.E], F32, tag="eqg", name="eqg")
                nc.gpsimd.tensor_scalar(smask[:], iota_g[:], gidxf[:, 0:1],
                                        None, op0=ALU.is_equal)
                for j in range(1, c.TKG):
                    nc.gpsimd.tensor_scalar(eqg[:], iota_g[:],
                                            gidxf[:, j:j + 1], None,
                                            op0=ALU.is_equal)
                    nc.gpsimd.tensor_tensor(smask[:], smask[:], eqg[:],
                                            op=ALU.add)
                masked = G2.tile([128, c.E], F32, tag="masked", name="masked")
                nc.vector.tensor_tensor(masked[:], exps[:], smask[:],
                                        op=ALU.mult)

                # top-K experts (sorted top-8, take first K)
                tkv = G2.tile([128, 8], F32, tag="tkv", name="tkv")
                tki = G2.tile([128, 8], U32, tag="tki", name="tki")
                nc.vector.max_with_indices(tkv[:], tki[:], masked[:])
                tkif = G2.tile([128, NK], F32, tag="tkif", name="tkif")
                nc.vector.tensor_copy(tkif[:], tki[:, :NK])
                tkvn = G2.tile([128, NK], F32, tag="tkvn", name="tkvn")
                nc.vector.tensor_scalar_mul(tkvn[:], tkv[:, :NK], rec[:])

                # weighted one-hot over local experts; one-hot = (woh > 0)
                ohL = oh_sb[:, tt, :]
                wohL = woh_sb[:, tt, :]
                weqL = G2.tile([128, c.EL], F32, tag="weqL", name="weqL")
                for k in range(NK):
                    if k == 0:
                        nc.vector.tensor_scalar(wohL, iota_eloc[:],
                                                tkif[:, 0:1], tkvn[:, 0:1],
                                                op0=ALU.is_equal,
                                                op1=ALU.mult)
                    else:
                        nc.vector.tensor_scalar(weqL[:], iota_eloc[:],
                                                tkif[:, k:k + 1],
                                                tkvn[:, k:k + 1],
                                                op0=ALU.is_equal,
                                                op1=ALU.mult)
                        nc.vector.tensor_tensor(wohL, wohL, weqL[:],
                                                op=ALU.add)
                nc.vector.tensor_scalar(ohL, wohL, 0.0, None, op0=ALU.is_gt)

                # positions: exclusive cumsum over tokens
                pos_ps = PST.tile([128, c.EL], F32, tag="pos_ps", name="pos_ps")
                nc.tensor.matmul(pos_ps[:], t1[:], ohL,
                                 start=True, stop=(tt == 0))
                for tp in range(tt):
                    nc.tensor.matmul(pos_ps[:], ones[:], oh_sb[:, tp, :],
                                     start=False, stop=(tp == tt - 1))
                pos = G2.tile([128, c.EL], F32, tag="pos", name="pos")
                nc.scalar.copy(pos[:], pos_ps[:])

                # masked positions for the slot compare
                tmp = G2.tile([128, c.EL], F32, tag="tmpA", name="tmpA")
                nc.vector.tensor_scalar(tmp[:], ohL, -float(BIGP),
                                        float(BIGP), op0=ALU.mult,
                                        op1=ALU.add)
                nc.vector.tensor_tensor(posm_sb[:, tt, :], pos[:], tmp[:],
                                        op=ALU.add)

                # combine indices, compacted: each token has at most EL
                # local slots; position j within the compacted list is the
                # exclusive cumsum of ohL.
                slot = G2.tile([128, c.EL], F32, tag="slot", name="slot")
                nc.vector.tensor_tensor(slot[:], pos[:], ebase[:],
                                        op=ALU.add)
                ovf = G2.tile([128, c.EL], F32, tag="ovf", name="ovf")
                nc.vector.tensor_scalar(ovf[:], pos[:], float(c.CAP),
                                        float(BIGP), op0=ALU.is_ge,
                                        op1=ALU.mult)
                nc.vector.tensor_tensor(slot[:], slot[:], ovf[:],
                                        op=ALU.add)
                nc.vector.tensor_scalar(slot[:], slot[:], float(c.NSLOT),
                                        -float(c.NSLOT), op0=ALU.min,
                                        op1=ALU.add)
                jp = G2.tile([128, c.EL], F32, tag="jp", name="jp")
                nc.gpsimd.memset(jp[:, 0:1], 0.0)
                nc.gpsimd.tensor_copy(jp[:, 1:2], ohL[:, 0:1])
                for el in range(2, c.EL):
                    nc.gpsimd.tensor_tensor(jp[:, el:el + 1],
                                            jp[:, el - 1:el],
                                            ohL[:, el - 1:el], op=ALU.add)
                oslot = G2.tile([128, c.EL], F32, tag="oslot", name="oslot")
                nc.vector.tensor_tensor(oslot[:], ohL, slot[:], op=ALU.mult)
                cidx = G2.tile([128, c.EL], F32, tag="cidx", name="cidx")
                eqc = G2.tile([128, c.EL], F32, tag="eqc", name="eqc")
                pr = G2.tile([128, c.EL], F32, tag="pr", name="pr")
                for j in range(c.EL):
                    nc.vector.tensor_scalar(eqc[:], jp[:], float(j), None,
                                            op0=ALU.is_equal)
                    nc.vector.tensor_tensor(pr[:], eqc[:], oslot[:],
                                            op=ALU.mult)
                    nc.vector.tensor_reduce(cidx[:, j:j + 1], pr[:], AX.X,
                                            ALU.add)
                nc.vector.tensor_scalar_add(cidx[:], cidx[:],
                                            float(c.NSLOT))

                ct_ps = PST.tile([c.EL, 128], F32, tag="ct_ps", name="ct_ps")
                nc.tensor.transpose(ct_ps[:], cidx[:], ident[:])
                ct16 = G2.tile([c.EL, 128], I16, tag="ct16", name="ct16")
                nc.vector.tensor_copy(ct16[:], ct_ps[:])
                dst = idxc_dram.ap()[tt * 128 * c.EL:(tt + 1) * 128 * c.EL]
                nc.sync.dma_start(
                    out=dst.rearrange("(t j) -> j t", j=c.EL), in_=ct16[:])
                for g in range(8):
                    nc.sync.dma_start(
                        out=idxc_sb[tt][g * 16:(g + 1) * 16, :],
                        in_=dst.rearrange("(f b) -> b f", b=16))

        # =================================================================
        # Phase B: slot->token inversion per local expert
        # =================================================================
        with tc.tile_pool(name="inv", bufs=2) as IV, \
                tc.tile_pool(name="ps_ids", bufs=2, space="PSUM") as PSI, \
                tc.tile_pool(name="ps_w", bufs=2, space="PSUM") as PSW, \
                tc.tile_pool(name="ps_wt", bufs=2, space="PSUM") as PSWT:
            for el in range(c.EL):
                ids_ps = PSI.tile([1, c.CAP], F32, tag="ids_ps", name="ids_ps")
                w_ps = PSW.tile([1, c.CAP], F32, tag="w_ps", name="w_ps")
                for tt in range(c.TT):
                    m = IV.tile([128, c.CAP], F32, tag="mcomp", name="mcomp")
                    nc.vector.tensor_scalar(m[:], iota_slot[:],
                                            posm_sb[:, tt, el:el + 1], None,
                                            op0=ALU.is_equal)
                    nc.tensor.matmul(ids_ps[:], iota_tok[:, tt:tt + 1], m[:],
                                     start=(tt == 0), stop=(tt == c.TT - 1))
                    nc.tensor.matmul(w_ps[:], woh_sb[:, tt, el:el + 1], m[:],
                                     start=(tt == 0), stop=(tt == c.TT - 1))
                idr = IV.tile([1, c.CAP], F32, tag="idr", name="idr")
                nc.vector.tensor_scalar(idr[:], ids_ps[:], -1.0, 0.0,
                                        op0=ALU.add, op1=ALU.max)
                id16 = IV.tile([1, c.CAP], I16, tag="id16", name="id16")
                nc.vector.tensor_copy(id16[:], idr[:])
                dst = idxd_dram.ap()[el * c.CAP:(el + 1) * c.CAP]
                nc.sync.dma_start(out=dst, in_=id16[:])
                for g in range(8):
                    nc.sync.dma_start(
                        out=idxd_sb[el][g * 16:(g + 1) * 16, :],
                        in_=dst.rearrange("(f b) -> b f", b=16))

                wrow = IV.tile([1, c.CAP], F32, tag="wrow", name="wrow")
                nc.scalar.activation(wrow[:], w_ps[:], AF.Copy,
                                     scale=c.SCALE)
                for sc in range(c.CAPC):
                    wt_ps = PSWT.tile([128, 1], F32, tag="wt_ps", name="wt_ps")
                    nc.tensor.transpose(
                        wt_ps[:], wrow[:, sc * 128:(sc + 1) * 128],
                        ident[:1, :1])
                    rank = el * c.CAPC + sc
                    nc.vector.tensor_copy(wslot_sb[:, rank:rank + 1],
                                          wt_ps[:])

        # =================================================================
        # Phase C: dispatch gather + expert MLPs
        # =================================================================
        with tc.tile_pool(name="w13p", bufs=c.HK + 4) as W13, \
                tc.tile_pool(name="w2p", bufs=c.IMK + 2) as W2P, \
                tc.tile_pool(name="xgp", bufs=2) as XGP, \
                tc.tile_pool(name="actp", bufs=2) as ACTP, \
                tc.tile_pool(name="ystp", bufs=3) as YST, \
                tc.tile_pool(name="sgp", bufs=2) as SGP, \
                tc.tile_pool(name="ps_gu", bufs=3, space="PSUM") as PSGU, \
                tc.tile_pool(name="ps_y", bufs=2, space="PSUM") as PSY:
            for el in range(c.EL):
                xg = XGP.tile([128, c.HK, c.CAP], BF16, tag="xg", name="xg")
                nc.gpsimd.dma_gather(
                    out_ap=xg[:], in_ap=xrow.ap(), idxs_ap=idxd_sb[el][:],
                    num_idxs=c.CAP, num_idxs_reg=c.CAP, elem_size=c.H,
                    transpose=True)

                w13k = []
                for kc in range(c.HK):
                    t = W13.tile([128, c.IM2], BF16, tag="w13t", name="w13t")
                    nc.sync.dma_start(
                        out=t[:],
                        in_=w13T.ap()[el, kc * 128:(kc + 1) * 128, :])
                    w13k.append(t)

                actT = ACTP.tile([128, c.IMK, c.CAP], BF16, tag="actT", name="actT")
                for mg in range(c.IMK):
                    gps = PSGU.tile([128, 512], F32, tag="gu_ps", name="gu_ps")
                    ups = PSGU.tile([128, 512], F32, tag="gu_ps", name="gu_ps")
                    for kc in range(c.HK):
                        nc.tensor.matmul(
                            gps[:, :c.CAP],
                            w13k[kc][:, mg * 128:(mg + 1) * 128],
                            xg[:, kc, :],
                            start=(kc == 0), stop=(kc == c.HK - 1))
                    for kc in range(c.HK):
                        nc.tensor.matmul(
                            ups[:, :c.CAP],
                            w13k[kc][:, (c.IMK + mg) * 128:
                                     (c.IMK + mg + 1) * 128],
                            xg[:, kc, :],
                            start=(kc == 0), stop=(kc == c.HK - 1))
                    sg = SGP.tile([128, c.CAP], F32, tag="sg", name="sg")
                    nc.scalar.activation(sg[:], gps[:, :c.CAP], AF.Sigmoid)
                    nc.vector.tensor_tensor(sg[:], sg[:], gps[:, :c.CAP],
                                            op=ALU.mult)
                    nc.vector.tensor_tensor(actT[:, mg, :], sg[:],
                                            ups[:, :c.CAP], op=ALU.mult)

                w2k = []
                for ic in range(c.IMK):
                    t = W2P.tile([128, c.H], BF16, tag="w2t", name="w2t")
                    nc.sync.dma_start(
                        out=t[:],
                        in_=w2T.ap()[el, ic * 128:(ic + 1) * 128, :])
                    w2k.append(t)

                for sc in range(c.CAPC):
                    rank = el * c.CAPC + sc
                    for hf in range(2):
                        y_ps = PSY.tile([128, HH], F32, tag="y_ps", name="y_ps")
                        for no, nh in _chunks(HH, 512):
                            for ic in range(c.IMK):
                                nc.tensor.matmul(
                                    y_ps[:, no:no + nh],
                                    actT[:, ic, sc * 128:(sc + 1) * 128],
                                    w2k[ic][:, hf * HH + no:hf * HH + no + nh],
                                    start=(ic == 0), stop=(ic == c.IMK - 1))
                        yst = YST.tile([128, HH], BF16, tag="yst", name="yst")
                        nc.scalar.activation(
                            yst[:], y_ps[:], AF.Copy,
                            scale=wslot_sb[:, rank:rank + 1])
                        nc.sync.dma_start(
                            out=y_dram.ap()[rank * 128:(rank + 1) * 128,
                                            hf * HH:(hf + 1) * HH],
                            in_=yst[:])

        # =================================================================
        # Phase D: shared-expert down proj + combine, per H half
        # =================================================================
        with tc.tile_pool(name="accp", bufs=1) as ACC, \
                tc.tile_pool(name="gthp", bufs=2) as GTH, \
                tc.tile_pool(name="tmpp", bufs=2) as TMP, \
                tc.tile_pool(name="ps_ysh", bufs=2, space="PSUM") as PSYS:
            for hf in range(2):
                acc = ACC.tile([128, HB, c.T], F32, tag="acc", name="acc")
                for hb in range(HB):
                    hc = hf * HB + hb
                    ysh = PSYS.tile([128, c.T], F32, tag="ysh_ps", name="ysh_ps")
                    for no, nh in _chunks(c.T, 512):
                        for gi, (sdt, at) in enumerate(
                                zip(sdn_tiles, actsh)):
                            nc.tensor.matmul(
                                ysh[:, no:no + nh],
                                sdt[:, hc * 128:(hc + 1) * 128],
                                at[:, no:no + nh],
                                start=(gi == 0),
                                stop=(gi == len(g_tiles) - 1))
                    nc.scalar.copy(acc[:, hb, :], ysh[:])

                nchunk = max(1, c.TT // 4)
                for tt in range(c.TT):
                    gt = GTH.tile([128, HB, 128 * c.EL], BF16, tag="gt", name="gt")
                    nc.gpsimd.dma_gather(
                        out_ap=gt[:],
                        in_ap=y_dram.ap()[:, hf * HH:(hf + 1) * HH],
                        idxs_ap=idxc_sb[tt][:],
                        num_idxs=128 * c.EL, num_idxs_reg=128 * c.EL,
                        elem_size=HH, elem_step=c.H, transpose=True)
                    red = TMP.tile([128, HB, 128], F32, tag="red", name="red")
                    nc.vector.tensor_reduce(
                        red[:],
                        gt[:].rearrange("p b (t j) -> p b t j", j=c.EL),
                        AX.X, ALU.add)
                    nc.vector.tensor_tensor(
                        acc[:, :, tt * 128:(tt + 1) * 128],
                        acc[:, :, tt * 128:(tt + 1) * 128],
                        red[:], op=ALU.add)
                    if (tt + 1) % nchunk == 0:
                        sl = slice((tt + 1 - nchunk) * 128, (tt + 1) * 128)
                        nc.sync.dma_start(
                            out=rs_in[hf].ap()
                                .rearrange("p (hc t) -> p hc t", t=c.T)[
                                    :, :, sl],
                            in_=acc[:, :, sl])

                if not c.no_collective:
                    nc.gpsimd.collective_compute(
                        "ReduceScatter", ALU.add,
                        ins=[rs_in[hf].ap().opt()],
                        outs=[rs_out[hf].ap().opt()],
                        replica_groups=[list(range(c.NC))],
                    )

        # =================================================================
        # Phase E: reduce-scatter + output
        # =================================================================
        OUTP = 128 // c.NC
        NB = c.NC                       # partition-widening factor
        FW2 = c.HK * c.T // c.NC // 2   # free width after widening, per half
        if c.no_collective:
            for hf in range(2):
                nc.sync.dma_start(
                    out=rs_out[hf].ap().rearrange("a (b f) -> (a b) f", b=NB),
                    in_=rs_in[hf].ap()[:OUTP, :]
                        .rearrange("a (b f) -> (a b) f", b=NB))
        with tc.tile_pool(name="outp", bufs=2) as OP:
            for hf in range(2):
                t = OP.tile([128, FW2], F32, tag="outt", name="outt")
                nc.sync.dma_start(
                    out=t[:],
                    in_=rs_out[hf].ap().rearrange("a (b f) -> (a b) f", b=NB))
                nc.sync.dma_start(
                    out=out_ext.ap()[:, hf * (c.HK * c.T // 2):
                                     (hf + 1) * (c.HK * c.T // 2)]
                        .rearrange("a (b f) -> a b f", b=NB),
                    in_=t[:])


# ---------------------------------------------------------------------------
# host side
# ---------------------------------------------------------------------------


def host_prep(cfg: Cfg, hidden_states, gate_w, w13, w2, shared_gu_w,
              shared_dn_w):
    c = cfg
    f32 = np.float32
    x = np.ascontiguousarray(np.asarray(hidden_states), dtype=f32)
    x_hi = x.astype(NPBF16)
    x_lo = (x - x_hi.astype(f32)).astype(NPBF16)
    gw = np.ascontiguousarray(np.asarray(gate_w), dtype=f32)
    gw_hi = gw.astype(NPBF16)
    gw_lo = (gw - gw_hi.astype(f32)).astype(NPBF16)

    pp = np.arange(128, dtype=f32)[:, None]
    com = {
        "xrow": np.ascontiguousarray(x_hi),
        "xhiT": np.ascontiguousarray(x_hi.T),
        "xloT": np.ascontiguousarray(x_lo.T),
        "gwhiT": np.ascontiguousarray(gw_hi.T),
        "gwloT": np.ascontiguousarray(gw_lo.T),
        "c_t1": (np.arange(128)[:, None] < np.arange(128)[None, :])
            .astype(f32),
        "c_ones": np.ones((128, 128), f32),
        "c_ident": np.eye(128, dtype=f32),
        "c_iota_tok": np.arange(c.TT, dtype=f32)[None, :] * 128 + pp + 1.0,
        "c_iota_slot": np.broadcast_to(
            np.arange(c.CAP, dtype=f32)[None, :], (128, c.CAP)).copy(),
        "c_iota_g": np.broadcast_to(
            (np.arange(c.E) // (c.E // c.G)).astype(f32)[None, :],
            (128, c.E)).copy(),
        "c_ebase": np.broadcast_to(
            (np.arange(c.EL, dtype=f32) * c.CAP)[None, :],
            (128, c.EL)).copy(),
    }

    w13 = np.asarray(w13); w2 = np.asarray(w2)
    shared_gu_w = np.asarray(shared_gu_w); shared_dn_w = np.asarray(shared_dn_w)

    in_maps = []
    for r in range(c.NC):
        m = dict(com)
        els = slice(r * c.EL, (r + 1) * c.EL)
        m["w13T"] = np.ascontiguousarray(
            np.transpose(w13[els].astype(f32), (0, 2, 1))).astype(NPBF16)
        m["w2T"] = np.ascontiguousarray(
            np.transpose(w2[els].astype(f32), (0, 2, 1))).astype(NPBF16)
        gsl = slice(r * c.ISHL, (r + 1) * c.ISHL)
        usl = slice(c.ISH + r * c.ISHL, c.ISH + (r + 1) * c.ISHL)
        sg = np.concatenate([shared_gu_w[gsl].astype(f32),
                             shared_gu_w[usl].astype(f32)], axis=0)
        m["sguT"] = np.ascontiguousarray(sg.T).astype(NPBF16)
        m["sdnT"] = np.ascontiguousarray(
            shared_dn_w[:, gsl].astype(f32).T).astype(NPBF16)
        m["c_iota_eloc"] = np.broadcast_to(
            (np.arange(c.EL, dtype=f32) + r * c.EL)[None, :],
            (128, c.EL)).copy()
        in_maps.append(m)
    return in_maps


def assemble(cfg: Cfg, results):
    # chunk r is [128/NC, 2, HK/2, T] with element (pp, hf, hcL, t) =
    # y^T[(hf*HK/2 + hcL)*128 + (128/NC)*r + pp, t]
    OUTP = 128 // cfg.NC
    st = np.stack([np.asarray(results[r]["out"], np.float32)
                   .reshape(OUTP, 2, cfg.HK // 2, cfg.T)
                   for r in range(cfg.NC)])            # [r, pp, hf, hcL, t]
    yT = np.transpose(st, (2, 3, 0, 1, 4)).reshape(cfg.H, cfg.T)
    return np.ascontiguousarray(yT.T)


_NC_CACHE = {}


def _get_nc(cfg: Cfg):
    if cfg not in _NC_CACHE:
        _NC_CACHE[cfg] = build_nc(cfg)
    return _NC_CACHE[cfg]


def kernel(**inputs) -> np.ndarray:
    from concourse.bass_utils import run_bass_kernel_spmd
    cfg = FULL
    nc = _get_nc(cfg)
    in_maps = host_prep(cfg, **inputs)
    res = run_bass_kernel_spmd(nc, in_maps, list(range(cfg.NC)))
    return assemble(cfg, res.results)



# revision 7
# speedup vs baseline: 1.6483x; 1.6483x over previous
"""Trainium2 Bass kernel for nn_CustomDeepseekDBOModel (DeepSeek-style MoE layer).

Strategy (8 NeuronCores, expert-parallel):
  * Every core receives the full token set plus its own shard of the
    routed-expert weights (4 of 32 experts) and a TP slice of the shared
    expert (1/8 of the intermediate dim).
  * The gate matrix rows are ROTATED per core (by r*EL experts = r groups)
    so each core's local experts are always score columns 0..3.  Group-
    limited top-k is permutation-invariant under whole-group rotation and
    is computed by iterative max+suppress thresholding -- batched over all
    8 token tiles in a handful of wide DVE ops (no per-token-tile chains).
  * Dispatch: per local expert, the token->slot map is inverted via tiny
    PE matmuls; token rows are fetched with a row-contiguous dma_gather
    (4KB descriptors) and transposed on the PE into [H-part, slot] tiles.
  * Expert SwiGLU MLP in bf16 produces y in [slot-part, H] layout kept in
    SBUF.  The combine is a PE matmul against a weighted one-hot matrix
    MT[slot, token] (built in 8 tensor_scalar ops), accumulated in the
    same PSUM as the shared-expert down-projection.
  * A bf16 ReduceScatter sums the (routed + shared-partial) transposed
    output across cores; each core emits its H/8 stripe.

kernel(**inputs) takes the FULL unsharded inputs and returns the full
[T, H] float32 output.
"""

from contextlib import ExitStack
from dataclasses import dataclass

import ml_dtypes
import numpy as np

import concourse.bass as bass  # noqa: F401
import concourse.mybir as mybir
import concourse.tile as tile
from concourse import bacc

F32 = mybir.dt.float32
BF16 = mybir.dt.bfloat16
I16 = mybir.dt.int16
NPBF16 = ml_dtypes.bfloat16

AF = mybir.ActivationFunctionType
ALU = mybir.AluOpType
AX = mybir.AxisListType


@dataclass(frozen=True)
class Cfg:
    T: int = 1024          # tokens
    H: int = 2048          # hidden
    E: int = 32            # routed experts
    K: int = 6             # top-k
    G: int = 8             # routing groups
    TKG: int = 3           # top-k groups
    IM: int = 768          # moe intermediate
    ISH: int = 1536        # shared intermediate (n_shared * IM)
    NC: int = 8            # cores
    CAP: int = 256         # per-expert capacity (multiple of 128)
    SCALE: float = 16.0
    no_collective: bool = False

    @property
    def EL(self):
        return self.E // self.NC

    @property
    def TT(self):
        return self.T // 128

    @property
    def HK(self):
        return self.H // 128

    @property
    def IM2(self):
        return 2 * self.IM

    @property
    def IMK(self):
        return self.IM // 128

    @property
    def CAPC(self):
        return self.CAP // 128

    @property
    def NSLOT(self):
        return self.EL * self.CAP

    @property
    def NBLK(self):
        return self.NSLOT // 128

    @property
    def ISHL(self):
        return self.ISH // self.NC

    @property
    def OUTP(self):
        return 128 // self.NC

    @property
    def HW2(self):
        return self.HK * self.T // 2


FULL = Cfg()

BIGP = float(1 << 14)  # suppression / mask constant


def _chunks(n, step=128):
    out = []
    o = 0
    while o < n:
        out.append((o, min(step, n - o)))
        o += step
    return out


# ---------------------------------------------------------------------------
# device program
# ---------------------------------------------------------------------------


def build_nc(cfg: Cfg):
    c = cfg
    nc = bacc.Bacc("TRN2", target_bir_lowering=False, debug=False,
                   num_devices=c.NC)

    def inp(name, shape, dt):
        return nc.dram_tensor(name, list(shape), dt, kind="ExternalInput")

    tn = {}
    tn["xrow"] = inp("xrow", (c.T, c.H), BF16)
    tn["xhiT"] = inp("xhiT", (c.H, c.T), BF16)
    tn["xloT"] = inp("xloT", (c.H, c.T), BF16)
    tn["gwhiT"] = inp("gwhiT", (c.H, c.E), BF16)
    tn["gwloT"] = inp("gwloT", (c.H, c.E), BF16)
    tn["w13T"] = inp("w13T", (c.EL, c.H, c.IM2), BF16)
    tn["w2T"] = inp("w2T", (c.EL, c.IM, c.H), BF16)
    tn["sguT"] = inp("sguT", (c.H, 2 * c.ISHL), BF16)
    tn["sdnT"] = inp("sdnT", (c.ISHL, c.H), BF16)
    tn["c_t1"] = inp("c_t1", (128, 128), F32)
    tn["c_ones"] = inp("c_ones", (128, 128), F32)
    tn["c_ident"] = inp("c_ident", (128, 128), F32)
    tn["c_identb"] = inp("c_identb", (128, 128), BF16)
    tn["c_iota_slot"] = inp("c_iota_slot", (128, c.CAP), F32)
    tn["c_iota_f"] = inp("c_iota_f", (128, c.T), F32)
    tn["c_iota_p1"] = inp("c_iota_p1", (128, 1), BF16)
    tn["c_128tt"] = inp("c_128tt", (128, c.TT), BF16)

    tn["out_ext"] = nc.dram_tensor("out", [2, c.OUTP, c.HW2], F32,
                                   kind="ExternalOutput")
    tn["rs_in0"] = nc.dram_tensor("rs_in0", [128, c.HW2], F32)
    tn["rs_in1"] = nc.dram_tensor("rs_in1", [128, c.HW2], F32)
    tn["rs_out0"] = nc.dram_tensor("rs_out0", [c.OUTP, c.HW2], F32)
    tn["rs_out1"] = nc.dram_tensor("rs_out1", [c.OUTP, c.HW2], F32)
    tn["idxd_dram"] = nc.dram_tensor("idxd_dram", [c.NSLOT], I16)

    with tile.TileContext(nc) as tc:
        _build_body(nc, tc, c, tn)
    nc.compile()
    return nc


def _build_body(nc, tc, c: Cfg, tn):
    xrow = tn["xrow"]; xhiT = tn["xhiT"]; xloT = tn["xloT"]
    gwhiT = tn["gwhiT"]; gwloT = tn["gwloT"]
    w13T = tn["w13T"]; w2T = tn["w2T"]; sguT = tn["sguT"]; sdnT = tn["sdnT"]
    rs_in = [tn["rs_in0"], tn["rs_in1"]]
    rs_out = [tn["rs_out0"], tn["rs_out1"]]
    out_ext = tn["out_ext"]
    idxd_dram = tn["idxd_dram"]

    HH = c.H // 2                  # H half
    HB = HH // 128                 # h-chunks per half

    with ExitStack() as es:
        P = es.enter_context(tc.tile_pool(name="persist", bufs=1))

        def load_const(t, shape, tag, dt=F32):
            tl = P.tile(list(shape), dt, tag=tag)
            nc.sync.dma_start(out=tl[:], in_=t.ap())
            return tl

        t1 = load_const(tn["c_t1"], (128, 128), "t1")
        ones = load_const(tn["c_ones"], (128, 128), "ones")
        ident = load_const(tn["c_ident"], (128, 128), "ident")
        identb = load_const(tn["c_identb"], (128, 128), "identb", BF16)
        iota_slot = load_const(tn["c_iota_slot"], (128, c.CAP), "iota_slot")
        iota_f = load_const(tn["c_iota_f"], (128, c.T), "iota_f")
        iota_p1 = load_const(tn["c_iota_p1"], (128, 1), "iota_p1", BF16)
        c128tt = load_const(tn["c_128tt"], (128, c.TT), "c128tt", BF16)

        # persistent outputs of the early phases
        posm = P.tile([128, c.TT, c.EL], F32, tag="posm", name="posm")
        pack4 = P.tile([128, c.TT, c.EL, 3], BF16, tag="pack4", name="pack4")
        idsT = P.tile([128, c.NBLK], F32, tag="idsT", name="idsT")
        wT = P.tile([128, c.NBLK], F32, tag="wT", name="wT")
        MT = P.tile([128, c.NBLK, c.T], BF16, tag="MT", name="MT")
        y_all = P.tile([128, c.NBLK, c.H], BF16, tag="y_all", name="y_all")
        idxd_sb = [P.tile([128, c.CAP // 16], I16, tag=f"idxd{el}",
                          name=f"idxd{el}") for el in range(c.EL)]
        g_tiles = _chunks(c.ISHL)
        sdn_tiles = []
        for gi, (ko, kh) in enumerate(g_tiles):
            t = P.tile([kh, c.H], BF16, tag=f"sdnt{gi}", name=f"sdnt{gi}")
            nc.sync.dma_start(out=t[:], in_=sdnT.ap()[ko:ko + kh, :])
            sdn_tiles.append(t)
        actsh = [P.tile([mh, c.T], BF16, tag=f"actsh{gi}", name=f"actsh{gi}")
                 for gi, (mo, mh) in enumerate(g_tiles)]
        lg_all = P.tile([128, c.TT, c.E], F32, tag="lg_all", name="lg_all")

        # =================================================================
        # gating logits + shared-expert gate/up (xhiT scope)
        # =================================================================
        with tc.tile_pool(name="xh", bufs=1) as XH:
            xhiT_k = []
            for kc in range(c.HK):
                t = XH.tile([128, c.T], BF16, tag=f"xhiT{kc}",
                            name=f"xhiT{kc}")
                nc.sync.dma_start(out=t[:],
                                  in_=xhiT.ap()[kc * 128:(kc + 1) * 128, :])
                xhiT_k.append(t)

            with tc.tile_pool(name="gate", bufs=1) as GP, \
                    tc.tile_pool(name="ps_gate", bufs=2, space="PSUM") as PSG, \
                    tc.tile_pool(name="ps_tp", bufs=1, space="PSUM") as PST:
                xloT_k = []
                for kc in range(c.HK):
                    t = GP.tile([128, c.T], BF16, tag=f"xloT{kc}",
                                name=f"xloT{kc}")
                    nc.sync.dma_start(
                        out=t[:], in_=xloT.ap()[kc * 128:(kc + 1) * 128, :])
                    xloT_k.append(t)
                gwhi_sb = GP.tile([128, c.HK, c.E], BF16, tag="gwhi",
                                  name="gwhi")
                nc.sync.dma_start(
                    out=gwhi_sb[:],
                    in_=gwhiT.ap().rearrange("(k p) e -> p k e", p=128))
                gwlo_sb = GP.tile([128, c.HK, c.E], BF16, tag="gwlo",
                                  name="gwlo")
                nc.sync.dma_start(
                    out=gwlo_sb[:],
                    in_=gwloT.ap().rearrange("(k p) e -> p k e", p=128))

                lgT = GP.tile([c.E, c.T], F32, tag="lgT", name="lgT")
                for no, nh in _chunks(c.T, 512):
                    ps = PSG.tile([c.E, 512], F32, tag="lgT_ps",
                                  name="lgT_ps")
                    for kc in range(c.HK):
                        pairs = [(gwhi_sb[:, kc, :], xhiT_k[kc]),
                                 (gwlo_sb[:, kc, :], xhiT_k[kc]),
                                 (gwhi_sb[:, kc, :], xloT_k[kc])]
                        for j, (lhsT, rhs) in enumerate(pairs):
                            nc.tensor.matmul(
                                ps[:, :nh], lhsT, rhs[:, no:no + nh],
                                start=(kc == 0 and j == 0),
                                stop=(kc == c.HK - 1 and j == 2))
                    nc.scalar.copy(lgT[:, no:no + nh], ps[:, :nh])

                # transpose logits to [token-part, tile, expert]
                lg_ps = PST.tile([128, c.TT, c.E], F32, tag="lg_ps",
                                 name="lg_ps")
                for tt in range(c.TT):
                    nc.tensor.transpose(
                        lg_ps[:, tt, :], lgT[:, tt * 128:(tt + 1) * 128],
                        ident[:c.E, :c.E])
                nc.scalar.copy(lg_all[:], lg_ps[:])

            # shared-expert gate/up: PE fills while DVE runs the routing
            with tc.tile_pool(name="sgw", bufs=1) as SGW, \
                    tc.tile_pool(name="ps_sh", bufs=2, space="PSUM") as PSSH, \
                    tc.tile_pool(name="sgshp", bufs=2) as SGSH:
                sgk = []
                for kc in range(c.HK):
                    t = SGW.tile([128, 2 * c.ISHL], BF16, tag=f"sgk{kc}",
                                 name=f"sgk{kc}")
                    nc.sync.dma_start(
                        out=t[:], in_=sguT.ap()[kc * 128:(kc + 1) * 128, :])
                    sgk.append(t)
                for gi, (mo, mh) in enumerate(g_tiles):
                    at = actsh[gi]
                    for no, nh in _chunks(c.T, 512):
                        gps = PSSH.tile([128, 512], F32, tag="gsh_ps",
                                        name="gsh_ps")
                        ups = PSSH.tile([128, 512], F32, tag="gsh_ps",
                                        name="gsh_ps")
                        for pso, tgt in ((mo, gps), (c.ISHL + mo, ups)):
                            for kc in range(c.HK):
                                nc.tensor.matmul(
                                    tgt[:mh, :nh],
                                    sgk[kc][:, pso:pso + mh],
                                    xhiT_k[kc][:, no:no + nh],
                                    start=(kc == 0), stop=(kc == c.HK - 1))
                        sg = SGSH.tile([128, 512], F32, tag="sgsh",
                                       name="sgsh")
                        nc.scalar.activation(sg[:mh, :nh], gps[:mh, :nh],
                                             AF.Sigmoid)
                        nc.vector.tensor_tensor(sg[:mh, :nh], sg[:mh, :nh],
                                                gps[:mh, :nh], op=ALU.mult)
                        nc.vector.tensor_tensor(at[:, no:no + nh],
                                                sg[:mh, :nh],
                                                ups[:mh, :nh], op=ALU.mult)

        # =================================================================
        # Phase A: batched routing (all 8 token tiles at once)
        # =================================================================
        def bc(t, shape):
            return t[:].unsqueeze(2).broadcast_to(shape)

        with tc.tile_pool(name="aphase", bufs=1) as A:
            exps = A.tile([128, c.TT, c.E], F32, tag="exps", name="exps")
            sums = A.tile([128, c.TT], F32, tag="sums", name="sums")
            rec = A.tile([128, c.TT], F32, tag="rec", name="rec")
            nc.scalar.activation(exps[:], lg_all[:], AF.Exp)
            nc.vector.tensor_reduce(sums[:], exps[:], AX.X, ALU.add)
            nc.vector.reciprocal(rec[:], sums[:])
            nc.vector.tensor_scalar(rec[:], rec[:], c.SCALE, None,
                                    op0=ALU.mult)

            # group scores: max over 4 experts per group
            gsc = A.tile([128, c.TT, c.G], F32, tag="gsc", name="gsc")
            nc.vector.tensor_reduce(
                gsc[:], exps[:].rearrange("p t (g r) -> p t g r", g=c.G),
                AX.X, ALU.max)

            # top-3 groups by iterative max+suppress; smask = gsc >= 3rd max
            gwork = A.tile([128, c.TT, c.G], F32, tag="gwork", name="gwork")
            gm = A.tile([128, c.TT], F32, tag="gm", name="gm")
            gsup = A.tile([128, c.TT, c.G], F32, tag="gsup", name="gsup")
            nc.vector.tensor_copy(gwork[:], gsc[:])
            for i in range(c.TKG - 1):
                nc.vector.tensor_reduce(gm[:], gwork[:], AX.X, ALU.max)
                nc.vector.tensor_tensor(gsup[:], gwork[:],
                                        bc(gm, [128, c.TT, c.G]),
                                        op=ALU.is_ge)
                nc.vector.tensor_scalar(gsup[:], gsup[:], -BIGP, None,
                                        op0=ALU.mult)
                nc.vector.tensor_tensor(gwork[:], gwork[:], gsup[:],
                                        op=ALU.add)
            nc.vector.tensor_reduce(gm[:], gwork[:], AX.X, ALU.max)
            smask = A.tile([128, c.TT, c.G], F32, tag="smask", name="smask")
            nc.vector.tensor_tensor(smask[:], gsc[:],
                                    bc(gm, [128, c.TT, c.G]), op=ALU.is_ge)

            # masked scores; top-6 experts by iterative max+suppress
            masked = A.tile([128, c.TT, c.E], F32, tag="masked",
                            name="masked")
            nc.vector.tensor_tensor(
                masked[:].rearrange("p t (g r) -> p t g r", g=c.G),
                exps[:].rearrange("p t (g r) -> p t g r", g=c.G),
                smask[:].unsqueeze(3).broadcast_to(
                    [128, c.TT, c.G, c.E // c.G]),
                op=ALU.mult)
            work2 = A.tile([128, c.TT, c.E], F32, tag="work2", name="work2")
            em = A.tile([128, c.TT], F32, tag="em", name="em")
            esup = A.tile([128, c.TT, c.E], F32, tag="esup", name="esup")
            nc.vector.tensor_copy(work2[:], masked[:])
            for i in range(c.K - 1):
                nc.vector.tensor_reduce(em[:], work2[:], AX.X, ALU.max)
                nc.vector.tensor_tensor(esup[:], work2[:],
                                        bc(em, [128, c.TT, c.E]),
                                        op=ALU.is_ge)
                nc.vector.tensor_scalar(esup[:], esup[:], -BIGP, None,
                                        op0=ALU.mult)
                nc.vector.tensor_tensor(work2[:], work2[:], esup[:],
                                        op=ALU.add)
            nc.vector.tensor_reduce(em[:], work2[:], AX.X, ALU.max)
            sel = A.tile([128, c.TT, c.E], F32, tag="sel", name="sel")
            nc.vector.tensor_tensor(sel[:], masked[:],
                                    bc(em, [128, c.TT, c.E]), op=ALU.is_ge)
            wsel = A.tile([128, c.TT, c.E], F32, tag="wsel", name="wsel")
            nc.vector.tensor_tensor(wsel[:], sel[:], exps[:], op=ALU.mult)
            nc.vector.tensor_tensor(wsel[:], wsel[:],
                                    bc(rec, [128, c.TT, c.E]), op=ALU.mult)

            # local experts are columns 0..EL-1 (gate rows rotated per core)
            ohL = sel[:, :, 0:c.EL]
            wohL = wsel[:, :, 0:c.EL]

            # positions: exclusive cumsum over tokens
            with tc.tile_pool(name="ps_pos", bufs=1, space="PSUM") as PSP:
                pos_ps = PSP.tile([128, c.TT, c.EL], F32, tag="pos_ps",
                                  name="pos_ps")
                for tt in range(c.TT):
                    nc.tensor.matmul(pos_ps[:, tt, :], t1[:], ohL[:, tt, :],
                                     start=True, stop=(tt == 0))
                    for tp in range(tt):
                        nc.tensor.matmul(pos_ps[:, tt, :], ones[:],
                                         ohL[:, tp, :],
                                         start=False, stop=(tp == tt - 1))
                pos_all = A.tile([128, c.TT, c.EL], F32, tag="pos_all",
                                 name="pos_all")
                nc.scalar.copy(pos_all[:], pos_ps[:])

            tmpm = A.tile([128, c.TT, c.EL], F32, tag="tmpm", name="tmpm")
            nc.vector.tensor_scalar(tmpm[:], ohL, -BIGP, BIGP, op0=ALU.mult,
                                    op1=ALU.add)
            nc.vector.tensor_tensor(posm[:], pos_all[:], tmpm[:], op=ALU.add)

            # lhsT pack for the slot-inversion matmuls: [p+1 | 128*tt | w]
            nc.vector.tensor_copy(
                pack4[:, :, :, 0],
                iota_p1[:].unsqueeze(2).broadcast_to([128, c.TT, c.EL]))
            nc.vector.tensor_copy(
                pack4[:, :, :, 1],
                c128tt[:].unsqueeze(2).broadcast_to([128, c.TT, c.EL]))
            nc.vector.tensor_copy(pack4[:, :, :, 2], wohL)

            # =============================================================
            # Phase B: slot->token inversion per local expert
            # =============================================================
            with tc.tile_pool(name="inv", bufs=2) as IV, \
                    tc.tile_pool(name="ps_idw", bufs=2, space="PSUM") as PSB, \
                    tc.tile_pool(name="ps_tr", bufs=1, space="PSUM") as PSTR:
                trI = PSTR.tile([128, c.NBLK], F32, tag="trI", name="trI")
                trW = PSTR.tile([128, c.NBLK], F32, tag="trW", name="trW")
                for el in range(c.EL):
                    mcomp = IV.tile([128, c.TT, c.CAP], BF16, tag="mcomp",
                                    name="mcomp")
                    for tt in range(c.TT):
                        nc.vector.tensor_scalar(
                            mcomp[:, tt, :], iota_slot[:],
                            posm[:, tt, el:el + 1], None, op0=ALU.is_equal)
                    ids_ps = PSB.tile([1, c.CAP], F32, tag="ids_ps",
                                      name="ids_ps")
                    w_ps = PSB.tile([1, c.CAP], F32, tag="w_ps",
                                    name="w_ps")
                    for tt in range(c.TT):
                        nc.tensor.matmul(ids_ps[:], pack4[:, tt, el, 0:1],
                                         mcomp[:, tt, :],
                                         start=(tt == 0), stop=False)
                        nc.tensor.matmul(ids_ps[:], pack4[:, tt, el, 1:2],
                                         mcomp[:, tt, :],
                                         start=False,
                                         stop=(tt == c.TT - 1))
                        nc.tensor.matmul(w_ps[:], pack4[:, tt, el, 2:3],
                                         mcomp[:, tt, :],
                                         start=(tt == 0),
                                         stop=(tt == c.TT - 1))
                    idr = IV.tile([1, c.CAP], F32, tag="idr", name="idr")
                    nc.vector.tensor_scalar(idr[:], ids_ps[:], -1.0, 0.0,
                                            op0=ALU.add, op1=ALU.max)
                    wrow = IV.tile([1, c.CAP], F32, tag="wrow", name="wrow")
                    nc.vector.tensor_copy(wrow[:], w_ps[:])
                    id16 = IV.tile([1, c.CAP], I16, tag="id16", name="id16")
                    nc.vector.tensor_copy(id16[:], idr[:])
                    dst = idxd_dram.ap()[el * c.CAP:(el + 1) * c.CAP]
                    nc.sync.dma_start(out=dst, in_=id16[:])
                    for g in range(8):
                        nc.sync.dma_start(
                            out=idxd_sb[el][g * 16:(g + 1) * 16, :],
                            in_=dst.rearrange("(f b) -> b f", b=16))
                    for sc in range(c.CAPC):
                        blk = el * c.CAPC + sc
                        nc.tensor.transpose(
                            trI[:, blk:blk + 1],
                            idr[:, sc * 128:(sc + 1) * 128], ident[:1, :1])
                        nc.tensor.transpose(
                            trW[:, blk:blk + 1],
                            wrow[:, sc * 128:(sc + 1) * 128],
                            ident[:1, :1])
                nc.vector.tensor_copy(idsT[:], trI[:])
                nc.vector.tensor_copy(wT[:], trW[:])

        # combine matrix MT[slot, token] = (token == ids[slot]) * w[slot]
        for blk in range(c.NBLK):
            nc.vector.tensor_scalar(MT[:, blk, :], iota_f[:],
                                    idsT[:, blk:blk + 1],
                                    wT[:, blk:blk + 1],
                                    op0=ALU.is_equal, op1=ALU.mult)

        # =================================================================
        # Phase C: dispatch gather + expert MLPs
        # =================================================================
        with tc.tile_pool(name="w13p", bufs=c.HK + 6) as W13, \
                tc.tile_pool(name="w2p", bufs=c.IMK + 2) as W2P, \
                tc.tile_pool(name="xgp", bufs=2) as XGP, \
                tc.tile_pool(name="xgtp", bufs=2) as XGT, \
                tc.tile_pool(name="actp", bufs=2) as ACTP, \
                tc.tile_pool(name="ps_tr2", bufs=1, space="PSUM") as PT, \
                tc.tile_pool(name="ps_gu", bufs=3, space="PSUM") as PSGU, \
                tc.tile_pool(name="ps_y", bufs=2, space="PSUM") as PSY:
            for el in range(c.EL):
                xg = XGP.tile([128, c.CAPC, c.H], BF16, tag="xg", name="xg")
                nc.gpsimd.dma_gather(
                    out_ap=xg[:], in_ap=xrow.ap(), idxs_ap=idxd_sb[el][:],
                    num_idxs=c.CAP, num_idxs_reg=c.CAP, elem_size=c.H,
                    transpose=False)

                # transpose gathered rows into [h-part, slot] layout
                xgT = XGT.tile([128, c.HK, c.CAP], BF16, tag="xgT",
                               name="xgT")
                for sc in range(c.CAPC):
                    for hq in range(c.HK // 4):
                        tp_ps = PT.tile([128, 4, 128], BF16, tag="tp_ps",
                                        name="tp_ps")
                        for j in range(4):
                            hc = hq * 4 + j
                            nc.tensor.transpose(
                                tp_ps[:, j, :],
                                xg[:, sc, hc * 128:(hc + 1) * 128],
                                identb[:])
                        nc.vector.tensor_copy(
                            xgT[:, hq * 4:(hq + 1) * 4,
                                sc * 128:(sc + 1) * 128], tp_ps[:])

                w13k = []
                for kc in range(c.HK):
                    t = W13.tile([128, c.IM2], BF16, tag="w13t", name="w13t")
                    nc.sync.dma_start(
                        out=t[:],
                        in_=w13T.ap()[el, kc * 128:(kc + 1) * 128, :])
                    w13k.append(t)

                actT = ACTP.tile([128, c.IMK, c.CAP], BF16, tag="actT",
                                 name="actT")
                for mg in range(c.IMK):
                    gps = PSGU.tile([128, c.CAP], F32, tag="gu_ps",
                                    name="gu_ps")
                    ups = PSGU.tile([128, c.CAP], F32, tag="gu_ps",
                                    name="gu_ps")
                    for kc in range(c.HK):
                        nc.tensor.matmul(
                            gps[:], w13k[kc][:, mg * 128:(mg + 1) * 128],
                            xgT[:, kc, :],
                            start=(kc == 0), stop=(kc == c.HK - 1))
                    for kc in range(c.HK):
                        nc.tensor.matmul(
                            ups[:],
                            w13k[kc][:, (c.IMK + mg) * 128:
                                     (c.IMK + mg + 1) * 128],
                            xgT[:, kc, :],
                            start=(kc == 0), stop=(kc == c.HK - 1))
                    sg = ACTP.tile([128, c.CAP], F32, tag="sg", name="sg")
                    nc.scalar.activation(sg[:], gps[:], AF.Sigmoid)
                    nc.vector.tensor_tensor(sg[:], sg[:], gps[:],
                                            op=ALU.mult)
                    nc.vector.tensor_tensor(actT[:, mg, :], sg[:], ups[:],
                                            op=ALU.mult)

                w2k = []
                for ic in range(c.IMK):
                    t = W2P.tile([128, c.H], BF16, tag="w2t", name="w2t")
                    nc.sync.dma_start(
                        out=t[:],
                        in_=w2T.ap()[el, ic * 128:(ic + 1) * 128, :])
                    w2k.append(t)

                for sc in range(c.CAPC):
                    blk = el * c.CAPC + sc
                    for hf in range(2):
                        y_ps = PSY.tile([128, HH], F32, tag="y_ps",
                                        name="y_ps")
                        for no, nh in _chunks(HH, 512):
                            for ic in range(c.IMK):
                                nc.tensor.matmul(
                                    y_ps[:, no:no + nh],
                                    actT[:, ic, sc * 128:(sc + 1) * 128],
                                    w2k[ic][:, hf * HH + no:
                                            hf * HH + no + nh],
                                    start=(ic == 0), stop=(ic == c.IMK - 1))
                        nc.scalar.copy(y_all[:, blk, hf * HH:(hf + 1) * HH],
                                       y_ps[:])

        # =================================================================
        # Phase D: combine + shared-expert down proj, fused in PSUM
        # =================================================================
        with tc.tile_pool(name="ysb", bufs=3) as YSB, \
                tc.tile_pool(name="ps_ysh", bufs=2, space="PSUM") as PSYS:
            for hf in range(2):
                for hb in range(HB):
                    hc = hf * HB + hb
                    ysh = PSYS.tile([128, c.T], F32, tag="ysh_ps",
                                    name="ysh_ps")
                    for no, nh in _chunks(c.T, 512):
                        for gi in range(len(g_tiles)):
                            nc.tensor.matmul(
                                ysh[:, no:no + nh],
                                sdn_tiles[gi][:, hc * 128:(hc + 1) * 128],
                                actsh[gi][:, no:no + nh],
                                start=(gi == 0), stop=False)
                        for blk in range(c.NBLK):
                            nc.tensor.matmul(
                                ysh[:, no:no + nh],
                                y_all[:, blk, hc * 128:(hc + 1) * 128],
                                MT[:, blk, no:no + nh],
                                start=False, stop=(blk == c.NBLK - 1))
                    ysb = YSB.tile([128, c.T], F32, tag="ysb", name="ysb")
                    nc.scalar.copy(ysb[:], ysh[:])
                    nc.sync.dma_start(
                        out=rs_in[hf].ap()[:, hb * c.T:(hb + 1) * c.T],
                        in_=ysb[:])
                if not c.no_collective:
                    nc.gpsimd.collective_compute(
                        "ReduceScatter", ALU.add,
                        ins=[rs_in[hf].ap().opt()],
                        outs=[rs_out[hf].ap().opt()],
                        replica_groups=[list(range(c.NC))],
                    )

        # =================================================================
        # Phase E: emit output stripes (SBUF bounce, bf16)
        # =================================================================
        with tc.tile_pool(name="outp", bufs=2) as OP:
            for hf in range(2):
                t = OP.tile([c.OUTP, c.HW2], F32, tag="outt", name="outt")
                if c.no_collective:
                    nc.sync.dma_start(out=t[:],
                                      in_=rs_in[hf].ap()[:c.OUTP, :])
                else:
                    nc.sync.dma_start(out=t[:], in_=rs_out[hf].ap())
                nc.sync.dma_start(out=out_ext.ap()[hf], in_=t[:])


# ---------------------------------------------------------------------------
# host side
# ---------------------------------------------------------------------------


def host_prep(cfg: Cfg, hidden_states, gate_w, w13, w2, shared_gu_w,
              shared_dn_w):
    c = cfg
    f32 = np.float32
    x = np.ascontiguousarray(np.asarray(hidden_states), dtype=f32)
    x_hi = x.astype(NPBF16)
    x_lo = (x - x_hi.astype(f32)).astype(NPBF16)
    gw = np.ascontiguousarray(np.asarray(gate_w), dtype=f32)

    pp = np.arange(128, dtype=f32)[:, None]
    com = {
        "xrow": np.ascontiguousarray(x_hi),
        "xhiT": np.ascontiguousarray(x_hi.T),
        "xloT": np.ascontiguousarray(x_lo.T),
        "c_t1": (np.arange(128)[:, None] < np.arange(128)[None, :])
            .astype(f32),
        "c_ones": np.ones((128, 128), f32),
        "c_ident": np.eye(128, dtype=f32),
        "c_identb": np.eye(128, dtype=f32).astype(NPBF16),
        "c_iota_slot": np.broadcast_to(
            np.arange(c.CAP, dtype=f32)[None, :], (128, c.CAP)).copy(),
        "c_iota_f": np.broadcast_to(
            np.arange(c.T, dtype=f32)[None, :], (128, c.T)).copy(),
        "c_iota_p1": (pp + 1.0).astype(NPBF16),
        "c_128tt": np.broadcast_to(
            (np.arange(c.TT, dtype=f32) * 128.0)[None, :],
            (128, c.TT)).copy().astype(NPBF16),
    }

    w13 = np.asarray(w13); w2 = np.asarray(w2)
    shared_gu_w = np.asarray(shared_gu_w)
    shared_dn_w = np.asarray(shared_dn_w)

    in_maps = []
    for r in range(c.NC):
        m = dict(com)
        # rotate experts so locals are always columns 0..EL-1; rotation by
        # whole groups (EL == E/G * ... == group size here) preserves the
        # group-limited routing structure.
        gwr = np.roll(gw, -r * c.EL, axis=0)
        gw_hi = gwr.astype(NPBF16)
        gw_lo = (gwr - gw_hi.astype(f32)).astype(NPBF16)
        m["gwhiT"] = np.ascontiguousarray(gw_hi.astype(f32).T) \
            .astype(NPBF16)
        m["gwloT"] = np.ascontiguousarray(gw_lo.astype(f32).T) \
            .astype(NPBF16)
        els = slice(r * c.EL, (r + 1) * c.EL)
        m["w13T"] = np.ascontiguousarray(
            np.transpose(w13[els].astype(f32), (0, 2, 1))).astype(NPBF16)
        m["w2T"] = np.ascontiguousarray(
            np.transpose(w2[els].astype(f32), (0, 2, 1))).astype(NPBF16)
        gsl = slice(r * c.ISHL, (r + 1) * c.ISHL)
        usl = slice(c.ISH + r * c.ISHL, c.ISH + (r + 1) * c.ISHL)
        sg = np.concatenate([shared_gu_w[gsl].astype(f32),
                             shared_gu_w[usl].astype(f32)], axis=0)
        m["sguT"] = np.ascontiguousarray(sg.T).astype(NPBF16)
        m["sdnT"] = np.ascontiguousarray(
            shared_dn_w[:, gsl].astype(f32).T).astype(NPBF16)
        in_maps.append(m)
    return in_maps


def assemble(cfg: Cfg, results):
    # results[r]["out"] is [2, OUTP, HW2] bf16 with element
    # (hf, pp, hb*T + t) = y^T[(hf*HB + hb)*128 + r*OUTP + pp, t]
    c = cfg
    HB = c.HK // 2
    st = np.stack([np.asarray(results[r]["out"])
                   .reshape(2, c.OUTP, HB, c.T).astype(np.float32)
                   for r in range(c.NC)])          # [r, hf, pp, hb, t]
    yT = np.transpose(st, (1, 3, 0, 2, 4)).reshape(c.H, c.T)
    return np.ascontiguousarray(yT.T)


_NC_CACHE = {}


def _get_nc(cfg: Cfg):
    if cfg not in _NC_CACHE:
        _NC_CACHE[cfg] = build_nc(cfg)
    return _NC_CACHE[cfg]


def kernel(**inputs) -> np.ndarray:
    from concourse.bass_utils import run_bass_kernel_spmd
    cfg = FULL
    nc = _get_nc(cfg)
    in_maps = host_prep(cfg, **inputs)
    res = run_bass_kernel_spmd(nc, in_maps, list(range(cfg.NC)))
    return assemble(cfg, res.results)


# revision 9
# speedup vs baseline: 1.6779x; 1.0179x over previous
"""Trainium2 Bass kernel for nn_CustomDeepseekDBOModel (DeepSeek-style MoE layer).

Strategy (8 NeuronCores, expert-parallel):
  * Every core receives the full token set plus its own shard of the
    routed-expert weights (4 of 32 experts) and a TP slice of the shared
    expert (1/8 of the intermediate dim).
  * The gate matrix rows are ROTATED per core (by r*EL experts = r groups)
    so each core's local experts are always score columns 0..3.  Group-
    limited top-k is permutation-invariant under whole-group rotation and
    is computed by iterative max+suppress thresholding -- batched over all
    8 token tiles in a handful of wide DVE ops (no per-token-tile chains).
  * Dispatch: per local expert, the token->slot map is inverted via tiny
    PE matmuls; token rows are fetched with a row-contiguous dma_gather
    (4KB descriptors) and transposed on the PE into [H-part, slot] tiles.
  * Expert SwiGLU MLP in bf16 produces y in [slot-part, H] layout kept in
    SBUF.  The combine is a PE matmul against a weighted one-hot matrix
    MT[slot, token] (built in 8 tensor_scalar ops), accumulated in the
    same PSUM as the shared-expert down-projection.
  * A bf16 ReduceScatter sums the (routed + shared-partial) transposed
    output across cores; each core emits its H/8 stripe.

kernel(**inputs) takes the FULL unsharded inputs and returns the full
[T, H] float32 output.
"""

from contextlib import ExitStack
from dataclasses import dataclass

import ml_dtypes
import numpy as np

import concourse.bass as bass  # noqa: F401
import concourse.mybir as mybir
import concourse.tile as tile
from concourse import bacc

F32 = mybir.dt.float32
BF16 = mybir.dt.bfloat16
I16 = mybir.dt.int16
NPBF16 = ml_dtypes.bfloat16

AF = mybir.ActivationFunctionType
ALU = mybir.AluOpType
AX = mybir.AxisListType


@dataclass(frozen=True)
class Cfg:
    T: int = 1024          # tokens
    H: int = 2048          # hidden
    E: int = 32            # routed experts
    K: int = 6             # top-k
    G: int = 8             # routing groups
    TKG: int = 3           # top-k groups
    IM: int = 768          # moe intermediate
    ISH: int = 1536        # shared intermediate (n_shared * IM)
    NC: int = 8            # cores
    CAP: int = 256         # per-expert capacity (multiple of 128)
    SCALE: float = 16.0
    no_collective: bool = False

    @property
    def EL(self):
        return self.E // self.NC

    @property
    def TT(self):
        return self.T // 128

    @property
    def HK(self):
        return self.H // 128

    @property
    def IM2(self):
        return 2 * self.IM

    @property
    def IMK(self):
        return self.IM // 128

    @property
    def CAPC(self):
        return self.CAP // 128

    @property
    def NSLOT(self):
        return self.EL * self.CAP

    @property
    def NBLK(self):
        return self.NSLOT // 128

    @property
    def ISHL(self):
        return self.ISH // self.NC

    @property
    def OUTP(self):
        return 128 // self.NC

    @property
    def HW2(self):
        return self.HK * self.T // 2


FULL = Cfg()

BIGP = float(1 << 14)  # suppression / mask constant


def _chunks(n, step=128):
    out = []
    o = 0
    while o < n:
        out.append((o, min(step, n - o)))
        o += step
    return out


# ---------------------------------------------------------------------------
# device program
# ---------------------------------------------------------------------------


def build_nc(cfg: Cfg):
    c = cfg
    nc = bacc.Bacc("TRN2", target_bir_lowering=False, debug=False,
                   num_devices=c.NC)

    def inp(name, shape, dt):
        return nc.dram_tensor(name, list(shape), dt, kind="ExternalInput")

    tn = {}
    tn["xrow"] = inp("xrow", (c.T, c.H), BF16)
    tn["xhiT"] = inp("xhiT", (c.H, c.T), BF16)
    tn["xloT"] = inp("xloT", (c.H, c.T), BF16)
    tn["gwhiT"] = inp("gwhiT", (c.H, c.E), BF16)
    tn["gwloT"] = inp("gwloT", (c.H, c.E), BF16)
    tn["w13T"] = inp("w13T", (c.EL, c.H, c.IM2), BF16)
    tn["w2T"] = inp("w2T", (c.EL, c.IM, c.H), BF16)
    tn["sguT"] = inp("sguT", (c.H, 2 * c.ISHL), BF16)
    tn["sdnT"] = inp("sdnT", (c.ISHL, c.H), BF16)
    tn["c_t1"] = inp("c_t1", (128, 128), F32)
    tn["c_ones"] = inp("c_ones", (128, 128), F32)
    tn["c_ident"] = inp("c_ident", (128, 128), F32)
    tn["c_identb"] = inp("c_identb", (128, 128), BF16)
    tn["c_iota_slot"] = inp("c_iota_slot", (128, c.CAP), F32)
    tn["c_iota_f"] = inp("c_iota_f", (128, c.T), F32)
    tn["c_iota_p1"] = inp("c_iota_p1", (128, 1), BF16)
    tn["c_128tt"] = inp("c_128tt", (128, c.TT), BF16)

    tn["out_ext"] = nc.dram_tensor("out", [2, c.OUTP, c.HW2], F32,
                                   kind="ExternalOutput")
    QW = c.HW2 // 2
    for q in range(4):
        tn[f"rs_in{q}"] = nc.dram_tensor(f"rs_in{q}", [128, QW], F32)
        tn[f"rs_out{q}"] = nc.dram_tensor(f"rs_out{q}", [c.OUTP, QW], F32)
    tn["idxd_dram"] = nc.dram_tensor("idxd_dram", [c.NSLOT], I16)

    with tile.TileContext(nc) as tc:
        _build_body(nc, tc, c, tn)
    nc.compile()
    return nc


def _build_body(nc, tc, c: Cfg, tn):
    xrow = tn["xrow"]; xhiT = tn["xhiT"]; xloT = tn["xloT"]
    gwhiT = tn["gwhiT"]; gwloT = tn["gwloT"]
    w13T = tn["w13T"]; w2T = tn["w2T"]; sguT = tn["sguT"]; sdnT = tn["sdnT"]
    rs_in = [tn[f"rs_in{q}"] for q in range(4)]
    rs_out = [tn[f"rs_out{q}"] for q in range(4)]
    out_ext = tn["out_ext"]
    idxd_dram = tn["idxd_dram"]

    HH = c.H // 2                  # H half
    HB = HH // 128                 # h-chunks per half

    with ExitStack() as es:
        P = es.enter_context(tc.tile_pool(name="persist", bufs=1))

        def load_const(t, shape, tag, dt=F32):
            tl = P.tile(list(shape), dt, tag=tag)
            nc.sync.dma_start(out=tl[:], in_=t.ap())
            return tl

        t1 = load_const(tn["c_t1"], (128, 128), "t1")
        ones = load_const(tn["c_ones"], (128, 128), "ones")
        ident = load_const(tn["c_ident"], (128, 128), "ident")
        identb = load_const(tn["c_identb"], (128, 128), "identb", BF16)
        iota_slot = load_const(tn["c_iota_slot"], (128, c.CAP), "iota_slot")
        iota_f = load_const(tn["c_iota_f"], (128, c.T), "iota_f")
        iota_p1 = load_const(tn["c_iota_p1"], (128, 1), "iota_p1", BF16)
        c128tt = load_const(tn["c_128tt"], (128, c.TT), "c128tt", BF16)

        # persistent outputs of the early phases
        posm = P.tile([128, c.TT, c.EL], F32, tag="posm", name="posm")
        pack4 = P.tile([128, c.TT, c.EL, 3], BF16, tag="pack4", name="pack4")
        idsT = P.tile([128, c.NBLK], F32, tag="idsT", name="idsT")
        wT = P.tile([128, c.NBLK], F32, tag="wT", name="wT")
        MT = P.tile([128, c.NBLK, c.T], BF16, tag="MT", name="MT")
        y_all = P.tile([128, c.NBLK, c.H], BF16, tag="y_all", name="y_all")
        idxd_sb = [P.tile([128, c.CAP // 16], I16, tag=f"idxd{el}",
                          name=f"idxd{el}") for el in range(c.EL)]
        g_tiles = _chunks(c.ISHL)
        sdn_tiles = []
        for gi, (ko, kh) in enumerate(g_tiles):
            t = P.tile([kh, c.H], BF16, tag=f"sdnt{gi}", name=f"sdnt{gi}")
            nc.sync.dma_start(out=t[:], in_=sdnT.ap()[ko:ko + kh, :])
            sdn_tiles.append(t)
        actsh = [P.tile([mh, c.T], BF16, tag=f"actsh{gi}", name=f"actsh{gi}")
                 for gi, (mo, mh) in enumerate(g_tiles)]
        lg_all = P.tile([128, c.TT, c.E], F32, tag="lg_all", name="lg_all")

        # =================================================================
        # gating logits + shared-expert gate/up (xhiT scope)
        # =================================================================
        with tc.tile_pool(name="xh", bufs=1) as XH:
            with tc.tile_pool(name="gate", bufs=1) as GP, \
                    tc.tile_pool(name="ps_gate", bufs=2, space="PSUM") as PSG, \
                    tc.tile_pool(name="ps_tp", bufs=1, space="PSUM") as PST:
                gwhi_sb = GP.tile([128, c.HK, c.E], BF16, tag="gwhi",
                                  name="gwhi")
                nc.sync.dma_start(
                    out=gwhi_sb[:],
                    in_=gwhiT.ap().rearrange("(k p) e -> p k e", p=128))
                gwlo_sb = GP.tile([128, c.HK, c.E], BF16, tag="gwlo",
                                  name="gwlo")
                nc.sync.dma_start(
                    out=gwlo_sb[:],
                    in_=gwloT.ap().rearrange("(k p) e -> p k e", p=128))
                xhiT_k = []
                xloT_k = []
                for kc in range(c.HK):
                    t = XH.tile([128, c.T], BF16, tag=f"xhiT{kc}",
                                name=f"xhiT{kc}")
                    nc.sync.dma_start(
                        out=t[:], in_=xhiT.ap()[kc * 128:(kc + 1) * 128, :])
                    xhiT_k.append(t)
                    t = GP.tile([128, c.T], BF16, tag=f"xloT{kc}",
                                name=f"xloT{kc}")
                    nc.sync.dma_start(
                        out=t[:], in_=xloT.ap()[kc * 128:(kc + 1) * 128, :])
                    xloT_k.append(t)

                lgT = GP.tile([c.E, c.T], F32, tag="lgT", name="lgT")
                for no, nh in _chunks(c.T, 512):
                    ps = PSG.tile([c.E, 512], F32, tag="lgT_ps",
                                  name="lgT_ps")
                    for kc in range(c.HK):
                        pairs = [(gwhi_sb[:, kc, :], xhiT_k[kc]),
                                 (gwlo_sb[:, kc, :], xhiT_k[kc]),
                                 (gwhi_sb[:, kc, :], xloT_k[kc])]
                        for j, (lhsT, rhs) in enumerate(pairs):
                            nc.tensor.matmul(
                                ps[:, :nh], lhsT, rhs[:, no:no + nh],
                                start=(kc == 0 and j == 0),
                                stop=(kc == c.HK - 1 and j == 2))
                    nc.scalar.copy(lgT[:, no:no + nh], ps[:, :nh])

                # transpose logits to [token-part, tile, expert]
                lg_ps = PST.tile([128, c.TT, c.E], F32, tag="lg_ps",
                                 name="lg_ps")
                for tt in range(c.TT):
                    nc.tensor.transpose(
                        lg_ps[:, tt, :], lgT[:, tt * 128:(tt + 1) * 128],
                        ident[:c.E, :c.E])
                nc.scalar.copy(lg_all[:], lg_ps[:])

            # shared-expert gate/up: PE fills while DVE runs the routing
            with tc.tile_pool(name="sgw", bufs=1) as SGW, \
                    tc.tile_pool(name="ps_sh", bufs=2, space="PSUM") as PSSH, \
                    tc.tile_pool(name="sgshp", bufs=2) as SGSH:
                sgk = []
                for kc in range(c.HK):
                    t = SGW.tile([128, 2 * c.ISHL], BF16, tag=f"sgk{kc}",
                                 name=f"sgk{kc}")
                    nc.sync.dma_start(
                        out=t[:], in_=sguT.ap()[kc * 128:(kc + 1) * 128, :])
                    sgk.append(t)
                for gi, (mo, mh) in enumerate(g_tiles):
                    at = actsh[gi]
                    for no, nh in _chunks(c.T, 512):
                        gps = PSSH.tile([128, 512], F32, tag="gsh_ps",
                                        name="gsh_ps")
                        ups = PSSH.tile([128, 512], F32, tag="gsh_ps",
                                        name="gsh_ps")
                        for pso, tgt in ((mo, gps), (c.ISHL + mo, ups)):
                            for kc in range(c.HK):
                                nc.tensor.matmul(
                                    tgt[:mh, :nh],
                                    sgk[kc][:, pso:pso + mh],
                                    xhiT_k[kc][:, no:no + nh],
                                    start=(kc == 0), stop=(kc == c.HK - 1))
                        sg = SGSH.tile([128, 512], F32, tag="sgsh",
                                       name="sgsh")
                        nc.scalar.activation(sg[:mh, :nh], gps[:mh, :nh],
                                             AF.Sigmoid)
                        nc.vector.tensor_tensor(sg[:mh, :nh], sg[:mh, :nh],
                                                gps[:mh, :nh], op=ALU.mult)
                        nc.vector.tensor_tensor(at[:, no:no + nh],
                                                sg[:mh, :nh],
                                                ups[:mh, :nh], op=ALU.mult)

        # =================================================================
        # Phase A: batched routing (all 8 token tiles at once)
        # =================================================================
        def bc(t, shape):
            return t[:].unsqueeze(2).broadcast_to(shape)

        with tc.tile_pool(name="aphase", bufs=1) as A:
            exps = A.tile([128, c.TT, c.E], F32, tag="exps", name="exps")
            sums = A.tile([128, c.TT], F32, tag="sums", name="sums")
            rec = A.tile([128, c.TT], F32, tag="rec", name="rec")
            nc.scalar.activation(exps[:], lg_all[:], AF.Exp)
            nc.vector.tensor_reduce(sums[:], exps[:], AX.X, ALU.add)
            nc.vector.reciprocal(rec[:], sums[:])
            nc.vector.tensor_scalar(rec[:], rec[:], c.SCALE, None,
                                    op0=ALU.mult)

            # group scores: max over 4 experts per group
            gsc = A.tile([128, c.TT, c.G], F32, tag="gsc", name="gsc")
            nc.vector.tensor_reduce(
                gsc[:], exps[:].rearrange("p t (g r) -> p t g r", g=c.G),
                AX.X, ALU.max)

            # top-3 groups: per-tile top-8 sort, threshold on 3rd value
            gv = A.tile([128, c.TT, 8], F32, tag="gv", name="gv")
            smask = A.tile([128, c.TT, c.G], F32, tag="smask", name="smask")
            for tt in range(c.TT):
                nc.vector.max(gv[:, tt, :], gsc[:, tt, :])
            for tt in range(c.TT):
                nc.vector.tensor_scalar(smask[:, tt, :], gsc[:, tt, :],
                                        gv[:, tt, c.TKG - 1:c.TKG], None,
                                        op0=ALU.is_ge)

            # masked scores; top-6 experts by iterative max+suppress
            masked = A.tile([128, c.TT, c.E], F32, tag="masked",
                            name="masked")
            nc.vector.tensor_tensor(
                masked[:].rearrange("p t (g r) -> p t g r", g=c.G),
                exps[:].rearrange("p t (g r) -> p t g r", g=c.G),
                smask[:].unsqueeze(3).broadcast_to(
                    [128, c.TT, c.G, c.E // c.G]),
                op=ALU.mult)
            ev = A.tile([128, c.TT, 8], F32, tag="ev", name="ev")
            sel = A.tile([128, c.TT, c.E], F32, tag="sel", name="sel")
            for tt in range(c.TT):
                nc.vector.max(ev[:, tt, :], masked[:, tt, :])
            for tt in range(c.TT):
                nc.vector.tensor_scalar(sel[:, tt, :], masked[:, tt, :],
                                        ev[:, tt, c.K - 1:c.K], None,
                                        op0=ALU.is_ge)
            wsel = A.tile([128, c.TT, c.E], F32, tag="wsel", name="wsel")
            nc.vector.tensor_tensor(wsel[:], sel[:], exps[:], op=ALU.mult)
            nc.vector.tensor_tensor(wsel[:], wsel[:],
                                    bc(rec, [128, c.TT, c.E]), op=ALU.mult)

            # local experts are columns 0..EL-1 (gate rows rotated per core)
            ohL = sel[:, :, 0:c.EL]
            wohL = wsel[:, :, 0:c.EL]

            # positions: exclusive cumsum over tokens
            with tc.tile_pool(name="ps_pos", bufs=1, space="PSUM") as PSP:
                pos_ps = PSP.tile([128, c.TT, c.EL], F32, tag="pos_ps",
                                  name="pos_ps")
                for tt in range(c.TT):
                    nc.tensor.matmul(pos_ps[:, tt, :], t1[:], ohL[:, tt, :],
                                     start=True, stop=(tt == 0))
                    for tp in range(tt):
                        nc.tensor.matmul(pos_ps[:, tt, :], ones[:],
                                         ohL[:, tp, :],
                                         start=False, stop=(tp == tt - 1))
                pos_all = A.tile([128, c.TT, c.EL], F32, tag="pos_all",
                                 name="pos_all")
                nc.scalar.copy(pos_all[:], pos_ps[:])

            tmpm = A.tile([128, c.TT, c.EL], F32, tag="tmpm", name="tmpm")
            nc.vector.tensor_scalar(tmpm[:], ohL, -BIGP, BIGP, op0=ALU.mult,
                                    op1=ALU.add)
            nc.vector.tensor_tensor(posm[:], pos_all[:], tmpm[:], op=ALU.add)

            # lhsT pack for the slot-inversion matmuls: [p+1 | 128*tt | w]
            nc.vector.tensor_copy(
                pack4[:, :, :, 0],
                iota_p1[:].unsqueeze(2).broadcast_to([128, c.TT, c.EL]))
            nc.vector.tensor_copy(
                pack4[:, :, :, 1],
                c128tt[:].unsqueeze(2).broadcast_to([128, c.TT, c.EL]))
            nc.vector.tensor_copy(pack4[:, :, :, 2], wohL)

            # =============================================================
            # Phase B: slot->token inversion per local expert
            # =============================================================
            with tc.tile_pool(name="inv", bufs=2) as IV, \
                    tc.tile_pool(name="ps_idw", bufs=2, space="PSUM") as PSB, \
                    tc.tile_pool(name="ps_tr", bufs=1, space="PSUM") as PSTR:
                trI = PSTR.tile([128, c.NBLK], F32, tag="trI", name="trI")
                trW = PSTR.tile([128, c.NBLK], F32, tag="trW", name="trW")
                for el in range(c.EL):
                    mcomp = IV.tile([128, c.TT, c.CAP], BF16, tag="mcomp",
                                    name="mcomp")
                    for tt in range(c.TT):
                        nc.vector.tensor_scalar(
                            mcomp[:, tt, :], iota_slot[:],
                            posm[:, tt, el:el + 1], None, op0=ALU.is_equal)
                    ids_ps = PSB.tile([1, c.CAP], F32, tag="ids_ps",
                                      name="ids_ps")
                    w_ps = PSB.tile([1, c.CAP], F32, tag="w_ps",
                                    name="w_ps")
                    for tt in range(c.TT):
                        nc.tensor.matmul(ids_ps[:], pack4[:, tt, el, 0:1],
                                         mcomp[:, tt, :],
                                         start=(tt == 0), stop=False)
                        nc.tensor.matmul(ids_ps[:], pack4[:, tt, el, 1:2],
                                         mcomp[:, tt, :],
                                         start=False,
                                         stop=(tt == c.TT - 1))
                        nc.tensor.matmul(w_ps[:], pack4[:, tt, el, 2:3],
                                         mcomp[:, tt, :],
                                         start=(tt == 0),
                                         stop=(tt == c.TT - 1))
                    idr = IV.tile([1, c.CAP], F32, tag="idr", name="idr")
                    nc.vector.tensor_scalar(idr[:], ids_ps[:], -1.0, 0.0,
                                            op0=ALU.add, op1=ALU.max)
                    wrow = IV.tile([1, c.CAP], F32, tag="wrow", name="wrow")
                    nc.vector.tensor_copy(wrow[:], w_ps[:])
                    id16 = IV.tile([1, c.CAP], I16, tag="id16", name="id16")
                    nc.vector.tensor_copy(id16[:], idr[:])
                    dst = idxd_dram.ap()[el * c.CAP:(el + 1) * c.CAP]
                    nc.sync.dma_start(out=dst, in_=id16[:])
                    for g in range(8):
                        nc.sync.dma_start(
                            out=idxd_sb[el][g * 16:(g + 1) * 16, :],
                            in_=dst.rearrange("(f b) -> b f", b=16))
                    for sc in range(c.CAPC):
                        blk = el * c.CAPC + sc
                        nc.tensor.transpose(
                            trI[:, blk:blk + 1],
                            idr[:, sc * 128:(sc + 1) * 128], ident[:1, :1])
                        nc.tensor.transpose(
                            trW[:, blk:blk + 1],
                            wrow[:, sc * 128:(sc + 1) * 128],
                            ident[:1, :1])
                nc.vector.tensor_copy(idsT[:], trI[:])
                nc.vector.tensor_copy(wT[:], trW[:])

        # combine matrix MT[slot, token] = (token == ids[slot]) * w[slot]
        for blk in range(c.NBLK):
            nc.vector.tensor_scalar(MT[:, blk, :], iota_f[:],
                                    idsT[:, blk:blk + 1],
                                    wT[:, blk:blk + 1],
                                    op0=ALU.is_equal, op1=ALU.mult)

        # =================================================================
        # Phase C: dispatch gather + expert MLPs
        # =================================================================
        with tc.tile_pool(name="w13p", bufs=c.HK + 6) as W13, \
                tc.tile_pool(name="w2p", bufs=c.IMK + 2) as W2P, \
                tc.tile_pool(name="xgp", bufs=2) as XGP, \
                tc.tile_pool(name="xgtp", bufs=2) as XGT, \
                tc.tile_pool(name="actp", bufs=2) as ACTP, \
                tc.tile_pool(name="ps_tr2", bufs=1, space="PSUM") as PT, \
                tc.tile_pool(name="ps_gu", bufs=3, space="PSUM") as PSGU, \
                tc.tile_pool(name="ps_y", bufs=2, space="PSUM") as PSY:
            for el in range(c.EL):
                xg = XGP.tile([128, c.CAPC, c.H], BF16, tag="xg", name="xg")
                nc.gpsimd.dma_gather(
                    out_ap=xg[:], in_ap=xrow.ap(), idxs_ap=idxd_sb[el][:],
                    num_idxs=c.CAP, num_idxs_reg=c.CAP, elem_size=c.H,
                    transpose=False)

                # transpose gathered rows into [h-part, slot] layout
                xgT = XGT.tile([128, c.HK, c.CAP], BF16, tag="xgT",
                               name="xgT")
                for sc in range(c.CAPC):
                    for hq in range(c.HK // 4):
                        tp_ps = PT.tile([128, 4, 128], BF16, tag="tp_ps",
                                        name="tp_ps")
                        for j in range(4):
                            hc = hq * 4 + j
                            nc.tensor.transpose(
                                tp_ps[:, j, :],
                                xg[:, sc, hc * 128:(hc + 1) * 128],
                                identb[:])
                        nc.vector.tensor_copy(
                            xgT[:, hq * 4:(hq + 1) * 4,
                                sc * 128:(sc + 1) * 128], tp_ps[:])

                w13k = []
                for kc in range(c.HK):
                    t = W13.tile([128, c.IM2], BF16, tag="w13t", name="w13t")
                    nc.sync.dma_start(
                        out=t[:],
                        in_=w13T.ap()[el, kc * 128:(kc + 1) * 128, :])
                    w13k.append(t)

                actT = ACTP.tile([128, c.IMK, c.CAP], BF16, tag="actT",
                                 name="actT")
                for mg in range(c.IMK):
                    gps = PSGU.tile([128, c.CAP], F32, tag="gu_ps",
                                    name="gu_ps")
                    ups = PSGU.tile([128, c.CAP], F32, tag="gu_ps",
                                    name="gu_ps")
                    for kc in range(c.HK):
                        nc.tensor.matmul(
                            gps[:], w13k[kc][:, mg * 128:(mg + 1) * 128],
                            xgT[:, kc, :],
                            start=(kc == 0), stop=(kc == c.HK - 1))
                    for kc in range(c.HK):
                        nc.tensor.matmul(
                            ups[:],
                            w13k[kc][:, (c.IMK + mg) * 128:
                                     (c.IMK + mg + 1) * 128],
                            xgT[:, kc, :],
                            start=(kc == 0), stop=(kc == c.HK - 1))
                    sg = ACTP.tile([128, c.CAP], F32, tag="sg", name="sg")
                    nc.scalar.activation(sg[:], gps[:], AF.Sigmoid)
                    nc.vector.tensor_tensor(sg[:], sg[:], gps[:],
                                            op=ALU.mult)
                    nc.vector.tensor_tensor(actT[:, mg, :], sg[:], ups[:],
                                            op=ALU.mult)

                w2k = []
                for ic in range(c.IMK):
                    t = W2P.tile([128, c.H], BF16, tag="w2t", name="w2t")
                    nc.sync.dma_start(
                        out=t[:],
                        in_=w2T.ap()[el, ic * 128:(ic + 1) * 128, :])
                    w2k.append(t)

                for sc in range(c.CAPC):
                    blk = el * c.CAPC + sc
                    for hf in range(2):
                        y_ps = PSY.tile([128, HH], F32, tag="y_ps",
                                        name="y_ps")
                        for no, nh in _chunks(HH, 512):
                            for ic in range(c.IMK):
                                nc.tensor.matmul(
                                    y_ps[:, no:no + nh],
                                    actT[:, ic, sc * 128:(sc + 1) * 128],
                                    w2k[ic][:, hf * HH + no:
                                            hf * HH + no + nh],
                                    start=(ic == 0), stop=(ic == c.IMK - 1))
                        nc.scalar.copy(y_all[:, blk, hf * HH:(hf + 1) * HH],
                                       y_ps[:])

        # =================================================================
        # Phase D: combine + shared-expert down proj, fused in PSUM
        # =================================================================
        with tc.tile_pool(name="ysb", bufs=3) as YSB, \
                tc.tile_pool(name="ps_ysh", bufs=2, space="PSUM") as PSYS:
            for hf in range(2):
                for hb in range(HB):
                    hc = hf * HB + hb
                    ysh = PSYS.tile([128, c.T], F32, tag="ysh_ps",
                                    name="ysh_ps")
                    for no, nh in _chunks(c.T, 512):
                        for gi in range(len(g_tiles)):
                            nc.tensor.matmul(
                                ysh[:, no:no + nh],
                                sdn_tiles[gi][:, hc * 128:(hc + 1) * 128],
                                actsh[gi][:, no:no + nh],
                                start=(gi == 0), stop=False)
                        for blk in range(c.NBLK):
                            nc.tensor.matmul(
                                ysh[:, no:no + nh],
                                y_all[:, blk, hc * 128:(hc + 1) * 128],
                                MT[:, blk, no:no + nh],
                                start=False, stop=(blk == c.NBLK - 1))
                    ysb = YSB.tile([128, c.T], F32, tag="ysb", name="ysb")
                    nc.scalar.copy(ysb[:], ysh[:])
                    q = hf * 2 + hb // (HB // 2)
                    hbq = hb % (HB // 2)
                    nc.sync.dma_start(
                        out=rs_in[q].ap()[:, hbq * c.T:(hbq + 1) * c.T],
                        in_=ysb[:])
                    if not c.no_collective and hbq == HB // 2 - 1:
                        nc.gpsimd.collective_compute(
                            "ReduceScatter", ALU.add,
                            ins=[rs_in[q].ap().opt()],
                            outs=[rs_out[q].ap().opt()],
                            replica_groups=[list(range(c.NC))],
                        )

        # =================================================================
        # Phase E: emit output stripes (SBUF bounce, bf16)
        # =================================================================
        QW = c.HW2 // 2
        with tc.tile_pool(name="outp", bufs=4) as OP:
            for q in range(4):
                t = OP.tile([c.OUTP, QW], F32, tag="outt", name="outt")
                if c.no_collective:
                    nc.sync.dma_start(out=t[:],
                                      in_=rs_in[q].ap()[:c.OUTP, :])
                else:
                    nc.sync.dma_start(out=t[:], in_=rs_out[q].ap())
                nc.sync.dma_start(
                    out=out_ext.ap()[q // 2][:, (q % 2) * QW:
                                             (q % 2 + 1) * QW],
                    in_=t[:])


# ---------------------------------------------------------------------------
# host side
# ---------------------------------------------------------------------------


def host_prep(cfg: Cfg, hidden_states, gate_w, w13, w2, shared_gu_w,
              shared_dn_w):
    c = cfg
    f32 = np.float32
    x = np.ascontiguousarray(np.asarray(hidden_states), dtype=f32)
    x_hi = x.astype(NPBF16)
    x_lo = (x - x_hi.astype(f32)).astype(NPBF16)
    gw = np.ascontiguousarray(np.asarray(gate_w), dtype=f32)

    pp = np.arange(128, dtype=f32)[:, None]
    com = {
        "xrow": np.ascontiguousarray(x_hi),
        "xhiT": np.ascontiguousarray(x_hi.T),
        "xloT": np.ascontiguousarray(x_lo.T),
        "c_t1": (np.arange(128)[:, None] < np.arange(128)[None, :])
            .astype(f32),
        "c_ones": np.ones((128, 128), f32),
        "c_ident": np.eye(128, dtype=f32),
        "c_identb": np.eye(128, dtype=f32).astype(NPBF16),
        "c_iota_slot": np.broadcast_to(
            np.arange(c.CAP, dtype=f32)[None, :], (128, c.CAP)).copy(),
        "c_iota_f": np.broadcast_to(
            np.arange(c.T, dtype=f32)[None, :], (128, c.T)).copy(),
        "c_iota_p1": (pp + 1.0).astype(NPBF16),
        "c_128tt": np.broadcast_to(
            (np.arange(c.TT, dtype=f32) * 128.0)[None, :],
            (128, c.TT)).copy().astype(NPBF16),
    }

    w13 = np.asarray(w13); w2 = np.asarray(w2)
    shared_gu_w = np.asarray(shared_gu_w)
    shared_dn_w = np.asarray(shared_dn_w)

    in_maps = []
    for r in range(c.NC):
        m = dict(com)
        # rotate experts so locals are always columns 0..EL-1; rotation by
        # whole groups (EL == E/G * ... == group size here) preserves the
        # group-limited routing structure.
        gwr = np.roll(gw, -r * c.EL, axis=0)
        gw_hi = gwr.astype(NPBF16)
        gw_lo = (gwr - gw_hi.astype(f32)).astype(NPBF16)
        m["gwhiT"] = np.ascontiguousarray(gw_hi.astype(f32).T) \
            .astype(NPBF16)
        m["gwloT"] = np.ascontiguousarray(gw_lo.astype(f32).T) \
            .astype(NPBF16)
        els = slice(r * c.EL, (r + 1) * c.EL)
        m["w13T"] = np.ascontiguousarray(
            np.transpose(w13[els].astype(f32), (0, 2, 1))).astype(NPBF16)
        m["w2T"] = np.ascontiguousarray(
            np.transpose(w2[els].astype(f32), (0, 2, 1))).astype(NPBF16)
        gsl = slice(r * c.ISHL, (r + 1) * c.ISHL)
        usl = slice(c.ISH + r * c.ISHL, c.ISH + (r + 1) * c.ISHL)
        sg = np.concatenate([shared_gu_w[gsl].astype(f32),
                             shared_gu_w[usl].astype(f32)], axis=0)
        m["sguT"] = np.ascontiguousarray(sg.T).astype(NPBF16)
        m["sdnT"] = np.ascontiguousarray(
            shared_dn_w[:, gsl].astype(f32).T).astype(NPBF16)
        in_maps.append(m)
    return in_maps


def assemble(cfg: Cfg, results):
    # results[r]["out"] is [2, OUTP, HW2] bf16 with element
    # (hf, pp, hb*T + t) = y^T[(hf*HB + hb)*128 + r*OUTP + pp, t]
    c = cfg
    HB = c.HK // 2
    st = np.stack([np.asarray(results[r]["out"])
                   .reshape(2, c.OUTP, HB, c.T).astype(np.float32)
                   for r in range(c.NC)])          # [r, hf, pp, hb, t]
    yT = np.transpose(st, (1, 3, 0, 2, 4)).reshape(c.H, c.T)
    return np.ascontiguousarray(yT.T)


_NC_CACHE = {}


def _get_nc(cfg: Cfg):
    if cfg not in _NC_CACHE:
        _NC_CACHE[cfg] = build_nc(cfg)
    return _NC_CACHE[cfg]


def kernel(**inputs) -> np.ndarray:
    from concourse.bass_utils import run_bass_kernel_spmd
    cfg = FULL
    nc = _get_nc(cfg)
    in_maps = host_prep(cfg, **inputs)
    res = run_bass_kernel_spmd(nc, in_maps, list(range(cfg.NC)))
    return assemble(cfg, res.results)
